# revision 1
# baseline (speedup 1.0000x reference)
"""Trainium2 Bass kernel for CRF negative-log-likelihood loss.

Problem: nn_CRF (B=512, L=1024, T=48), data-parallel over 8 NeuronCores
(64 batch rows per core). Each core computes a scalar partial loss; the
host sums the 8 partials.

Per-core algorithm (validated against a float64 numpy reference):
  forward (partition function):
    exp-domain scan A_t[j,b] = sum_i E[i,j] A_{t-1}[i,b] * F_t[j,b]
    with E = exp(trans - log T) as stationary PE weights extended with an
    exp(end) capture column and a ones colsum column; F_t = exp(feat_t - MU)
    produced by bulk PE transposes + fused ACT exp-copies. Per-b
    renormalization every R steps is folded into the F tile DELTA steps
    later (off the critical path); log-scales accumulate via the
    suffix-mask identity sum_t ind[t,b]*logS(t)[b] =
    sum_rho log s_rho[b] * maskT[apply_rho][b]. The mask never enters the
    scan: terminal alphas are recovered by indicator-selection
    (ind = maskT[t] - maskT[t+1]) over captured end-rows.
  gold (numerator): one-hot tiles via per-partition tag scalars
    (tensor_scalar is_equal), a bigram-count matmul C = OHu^T @ OHm_shift
    accumulated in PSUM then contracted with trans, and fused
    tensor_tensor_reduce feat gathers. Everything reduces through
    ones-matmuls into PSUM scalar accumulators.
"""

import math

import numpy as np

import concourse.bacc as bacc
import concourse.mybir as mybir
import concourse.tile as tile
from concourse.bass_utils import run_bass_kernel_spmd

F32 = mybir.dt.float32
I32 = mybir.dt.int32
AF = mybir.ActivationFunctionType
OP = mybir.AluOpType

B_FULL = 512
N_CORES = 8
BC = B_FULL // N_CORES  # 64
L_FULL = 1024
T = 48

MU = 0.51                # per-step feat shift folded into F (calibrated
                         # so mean per-step log-gain ~ 0: keeps Ln inputs
                         # inside the ACT spline accurate range)
A_SHIFT = math.log(T)    # shift folded into E
R = 16                   # renorm period (steps)
DELTA = 8                # renorm application delay (steps)
FCHUNK = 32              # timesteps per F-prep DMA chunk


def build_program(L=L_FULL, Bc=BC, G=1, dbg=False):
    """Emit the full per-core program; returns the compiled Bacc object."""
    assert L % 128 == 0 and L % FCHUNK == 0
    Nb = Bc // G
    CAP0 = L // 2          # captures kept for t >= CAP0-1 (lengths >= L/2)
    assert CAP0 % 128 == 0
    n_tt = L // 128
    n_cap = (L - CAP0) // 128
    nchunks = L // FCHUNK

    nc = bacc.Bacc("TRN2", target_bir_lowering=False, debug=False)

    feats_d = nc.dram_tensor("feats", (Bc, L, T), F32, kind="ExternalInput")
    trans_d = nc.dram_tensor("trans", (T, T), F32, kind="ExternalInput")
    start_d = nc.dram_tensor("start", (T,), F32, kind="ExternalInput")
    end_d = nc.dram_tensor("end", (T,), F32, kind="ExternalInput")
    tags_d = nc.dram_tensor("tags", (Bc, L), I32, kind="ExternalInput")
    mask_d = nc.dram_tensor("mask", (Bc, L), I32, kind="ExternalInput")
    out_d = nc.dram_tensor("out", (1, 1), F32, kind="ExternalOutput")
    dbg_d = (nc.dram_tensor("dbg", (6, Bc), F32, kind="ExternalOutput")
             if dbg else None)

    feats_flat = feats_d.ap().rearrange("b l t -> b (l t)")

    # renorm schedule: at MM step t (t % R == 0, t+DELTA-1 < L) the colsum of
    # A_{t-1} is available; its reciprocal is folded into F at t-1+DELTA.
    renorm_ts = [t for t in range(R, L + 1, R) if t + DELTA - 1 < L]

    with tile.TileContext(nc) as tc:
        with (
            tc.tile_pool(name="const", bufs=1) as cp,
            tc.tile_pool(name="cpsum", bufs=1, space="PSUM") as cpp,
        ):
            # ---------------- constants ----------------
            iota48i = cp.tile((128, T), I32)
            nc.gpsimd.iota(iota48i[:, :], [[1, T]], channel_multiplier=0)
            iota48f = cp.tile((128, T), F32)
            nc.vector.tensor_copy(iota48f[:, :], iota48i[:, :])

            iotaLi = cp.tile((Bc, L), I32)
            nc.gpsimd.iota(iotaLi[:, :], [[1, L]], channel_multiplier=0)
            iotaLf = cp.tile((Bc, L), F32)
            nc.vector.tensor_copy(iotaLf[:, :], iotaLi[:, :])

            iota64i = cp.tile((64, 64), I32)
            nc.gpsimd.iota(iota64i[:, :], [[1, 64]], channel_multiplier=0)
            iotaPi = cp.tile((64, 1), I32)
            nc.gpsimd.iota(iotaPi[:, :], [[1, 1]], channel_multiplier=1)
            iota64f = cp.tile((64, 64), F32)
            nc.vector.tensor_copy(iota64f[:, :], iota64i[:, :])
            iotaPf = cp.tile((64, 1), F32)
            nc.vector.tensor_copy(iotaPf[:, :], iotaPi[:, :])
            identM = cp.tile((64, 64), F32)
            nc.vector.tensor_scalar(
                identM[:, :], iota64f[:, :], iotaPf[:, :], None, OP.is_equal)

            ones128 = cp.tile((128, 1), F32)
            nc.vector.memset(ones128[:, :], 1.0)
            onesrow = cp.tile((1, T), F32)
            nc.vector.memset(onesrow[:, :], 1.0)

            # activation bias tiles (arbitrary float biases need APs)
            bias_a = cp.tile((T, 1), F32)
            nc.vector.memset(bias_a[:, :], -A_SHIFT)
            bias_mu = cp.tile((T, 1), F32)
            nc.vector.memset(bias_mu[:, :], -MU)

            # ---------------- params ----------------
            trans_sb = cp.tile((T, T), F32)
            nc.sync.dma_start(trans_sb[:, :], trans_d.ap())
            e_mat = cp.tile((T, T), F32)
            nc.scalar.activation(e_mat[:, :], trans_sb[:, :], AF.Exp,
                                 bias=bias_a[:, :])
            end_sb = cp.tile((T, 1), F32)
            nc.sync.dma_start(end_sb[:, :], end_d.ap().unsqueeze(1))
            expend = cp.tile((T, 1), F32)
            nc.scalar.activation(expend[:, :], end_sb[:, :], AF.Exp)
            ones48c = cp.tile((T, 1), F32)
            nc.vector.memset(ones48c[:, :], 1.0)

            start_sb = cp.tile((T, 1), F32)
            nc.sync.dma_start(start_sb[:, :],
                              start_d.ap().unsqueeze(1))
            expstart = cp.tile((T, 1), F32)
            nc.scalar.activation(expstart[:, :], start_sb[:, :], AF.Exp)

            startbc = cp.tile((Bc, T), F32)
            nc.sync.dma_start(
                startbc[:, :],
                start_d.ap().unsqueeze(0).partition_broadcast(Bc))
            endbc = cp.tile((Bc, T), F32)
            nc.sync.dma_start(
                endbc[:, :],
                end_d.ap().unsqueeze(0).partition_broadcast(Bc))

            # ---------------- tags / mask ----------------
            tags_i = cp.tile((Bc, L), I32)
            nc.sync.dma_start(tags_i[:, :], tags_d.ap())
            tagsf = cp.tile((Bc, L), F32)
            nc.vector.tensor_copy(tagsf[:, :], tags_i[:, :])
            mask_i = cp.tile((Bc, L), I32)
            nc.sync.dma_start(mask_i[:, :], mask_d.ap())
            maskf = cp.tile((Bc, L), F32)
            nc.vector.tensor_copy(maskf[:, :], mask_i[:, :])

            # transposed (128-timestep x Bc) tag/mask tiles
            prep_scope = tc.tile_pool(name="prepps", bufs=2, space="PSUM")
            ppp = prep_scope.__enter__()
            maskT = []
            tagsT = []
            for k in range(n_tt):
                ps = ppp.tile((128, Bc), F32, name=f"tp_ps_{k}", tag="tp_ps",
                              bufs=2)
                nc.tensor.transpose(ps[:, :], maskf[:, 128 * k:128 * (k + 1)],
                                    identM[:, :])
                mt = cp.tile((128, Bc), F32, name=f"maskT_{k}")
                nc.scalar.copy(mt[:, :], ps[:, :])
                maskT.append(mt)
                ps2 = ppp.tile((128, Bc), F32, name=f"tp_ps2_{k}",
                               tag="tp_ps", bufs=2)
                nc.tensor.transpose(ps2[:, :], tagsf[:, 128 * k:128 * (k + 1)],
                                    identM[:, :])
                tt = cp.tile((128, Bc), F32, name=f"tagsT_{k}")
                nc.scalar.copy(tt[:, :], ps2[:, :])
                tagsT.append(tt)

            # shifted (t+1) variants via partition-shift DMAs
            zero_row = cp.tile((1, Bc), F32)
            nc.vector.memset(zero_row[:, :], 0.0)
            maskTs = []
            tagsTs = []
            for k in range(n_tt):
                ms = cp.tile((128, Bc), F32, name=f"maskTs_{k}")
                nc.sync.dma_start(ms[0:127, :], maskT[k][1:128, :])
                ts_ = cp.tile((128, Bc), F32, name=f"tagsTs_{k}")
                nc.sync.dma_start(ts_[0:127, :], tagsT[k][1:128, :])
                if k + 1 < n_tt:
                    nc.sync.dma_start(ms[127:128, :], maskT[k + 1][0:1, :])
                    nc.sync.dma_start(ts_[127:128, :], tagsT[k + 1][0:1, :])
                else:
                    nc.sync.dma_start(ms[127:128, :], zero_row[:, :])
                    nc.sync.dma_start(ts_[127:128, :], zero_row[:, :])
                maskTs.append(ms)
                tagsTs.append(ts_)

            # masked tag tiles: tag + (1-mask)*100 makes the one-hot vanish
            tagsTm = []
            tagsTsm = []
            for k in range(n_tt):
                off = cp.tile((128, Bc), F32, name=f"moff_{k}")
                nc.vector.tensor_scalar(off[:, :], maskT[k][:, :], -100.0,
                                        100.0, OP.mult, OP.add)
                tm = cp.tile((128, Bc), F32, name=f"tagsTm_{k}")
                nc.vector.tensor_tensor(tm[:, :], tagsT[k][:, :], off[:, :],
                                        OP.add)
                tagsTm.append(tm)
                offs = cp.tile((128, Bc), F32, name=f"moffs_{k}")
                nc.vector.tensor_scalar(offs[:, :], maskTs[k][:, :], -100.0,
                                        100.0, OP.mult, OP.add)
                tms = cp.tile((128, Bc), F32, name=f"tagsTsm_{k}")
                nc.vector.tensor_tensor(tms[:, :], tagsTs[k][:, :],
                                        offs[:, :], OP.add)
                tagsTsm.append(tms)

            # indicator ind[t,b] = maskT[t] - maskT[t+1] (last row: maskT)
            ind = []
            for k in range(n_tt):
                it = cp.tile((128, Bc), F32, name=f"ind_{k}")
                nc.vector.tensor_tensor(it[:, :], maskT[k][:, :],
                                        maskTs[k][:, :], OP.subtract)
                ind.append(it)
            ind_c0 = cp.tile((1, Bc), F32)
            nc.sync.dma_start(ind_c0[:, :], ind[CAP0 // 128 - 1][127:128, :])

            # partition-0-aligned mask rows for each renorm fold time
            mrow = {}
            for t in renorm_ts:
                tf = t - 1 + DELTA
                mr = cp.tile((1, Bc), F32, name=f"mrow_{tf}")
                nc.sync.dma_start(mr[:, :],
                                  maskT[tf // 128][tf % 128:tf % 128 + 1, :])
                mrow[t] = mr

            # len row (1, Bc) via ones-matmul over maskT tiles
            len_ps = ppp.tile((1, Bc), F32, name="len_ps", tag="len_ps",
                              bufs=1)
            for k in range(n_tt):
                nc.tensor.matmul(len_ps[:, :], ones128[:, :], maskT[k][:, :],
                                 start=(k == 0), stop=(k == n_tt - 1),
                                 skip_group_check=True)
            lenm1_row = cp.tile((1, Bc), F32)
            nc.vector.tensor_scalar(lenm1_row[:, :], len_ps[:, :], 1.0, None,
                                    OP.subtract)
            prep_scope.__exit__(None, None, None)

            # persistent accumulators
            logsel = cp.tile((1, Bc), F32)
            nc.vector.memset(logsel[:, :], 0.0)
            feat_acc = cp.tile((128, Bc * n_tt), F32)
            misc_acc = cp.tile((Bc, 4), F32)

            c_ps = cpp.tile((T, T), F32, name="c_ps")  # bigram counts

            # =============== scan + F-prep + gold ===============
            # capture staging lives in DRAM: one row per captured step,
            # packed contiguously; split into (t, b) tiles in the end phase.
            ncap_steps = L - (CAP0 - 8)
            with tc.tile_pool(name="dramp", bufs=1, space="DRAM") as dp:
                cap_stage = dp.tile((1, ncap_steps * Bc), F32,
                                    name="cap_stage")
            with (
                tc.tile_pool(name="natp", bufs=3) as natp,
                tc.tile_pool(name="fpool", bufs=10) as fpool,
                tc.tile_pool(name="tpps", bufs=2, space="PSUM") as tpps,
                tc.tile_pool(name="scanps", bufs=1, space="PSUM") as scanps,
                tc.tile_pool(name="capps", bufs=2, space="PSUM") as capps,
                tc.tile_pool(name="rbcps", bufs=1, space="PSUM") as rbcps,
                tc.tile_pool(name="csps", bufs=1, space="PSUM") as csps,
                tc.tile_pool(name="apool", bufs=3) as apool,
                tc.tile_pool(name="fgp", bufs=6) as fgp,
                tc.tile_pool(name="ohp", bufs=8) as ohp,
                tc.tile_pool(name="scrp", bufs=2) as scrp,
            ):
                ftiles = {}

                def emit_fprep(c):
                    # one chunk = FCHUNK timesteps; F tiles hold 8 t each
                    natf = natp.tile((Bc, FCHUNK * T), F32, name="natf")
                    nc.sync.dma_start(
                        natf[:, :],
                        feats_flat[:, FCHUNK * T * c:FCHUNK * T * (c + 1)])
                    for q in range(FCHUNK // 8):
                        ps = tpps.tile((T, 512), F32, name="tp")
                        for k in range(8):
                            blk = q * 8 + k
                            nc.tensor.transpose(
                                ps[:, 64 * k:64 * k + Bc],
                                natf[:, T * blk:T * (blk + 1)],
                                identM[:, :])
                        ft = fpool.tile((T, 512), F32, name="ftile")
                        nc.scalar.activation(ft[:, :], ps[:, :], AF.Exp,
                                             bias=bias_mu[:, :])
                        ftiles[c * (FCHUNK // 8) + q] = ft

                def f_slice(t, g=0):
                    ft = ftiles[t // 8]
                    c0 = (t % 8) * 64
                    return ft[0:T, c0 + g * Nb:c0 + (g + 1) * Nb]

                emit_fprep(0)
                emit_fprep(1)

                # A0 = exp(start) * F_0
                a_prev = apool.tile((T, Bc), F32, name="a_t")
                nc.vector.tensor_scalar(
                    a_prev[:, :], ftiles[0][0:T, 0:Bc], expstart[:, :],
                    None, OP.mult)

                for t in range(1, L + 1):
                    if t % FCHUNK == 1:
                        c = (t - 1) // FCHUNK + 2
                        if c < nchunks:
                            emit_fprep(c)
                    tprev = t - 1
                    # end-capture of A_{t-1}: ring row in PSUM, flushed to
                    # SBUF staging by ACT once per 8 steps
                    if tprev >= CAP0 - 8:
                        slot = (tprev - (CAP0 - 8)) % 8
                        if slot == 0:
                            cap_ring = capps.tile((1, 8 * Bc), F32,
                                                  name="cap_ring")
                        nc.tensor.matmul(
                            cap_ring[0:1, slot * Bc:(slot + 1) * Bc],
                            expend[:, :], a_prev[:, :],
                            start=True, stop=True, skip_group_check=True)
                        if slot == 7:
                            blk = (tprev - (CAP0 - 8)) // 8
                            crow = scrp.tile((1, 8 * Bc), F32, name="crow",
                                             tag="crow")
                            nc.scalar.copy(crow[0:1, :], cap_ring[0:1, :])
                            nc.sync.dma_start(
                                cap_stage[0:1, blk * 8 * Bc:
                                          (blk + 1) * 8 * Bc],
                                crow[0:1, :])
                    # renorm: colsum of A_{t-1} via ones-matmul, fold at t-1+DELTA
                    if t in mrow:
                        tf = t - 1 + DELTA
                        for g in range(G):
                            gs = slice(g * Nb, (g + 1) * Nb)
                            cs = csps.tile((1, Nb), F32, name="cs")
                            nc.tensor.matmul(
                                cs[:, :], ones48c[:, :], a_prev[:, gs],
                                start=True, stop=True, skip_group_check=True)
                            r_sb = scrp.tile((1, Nb), F32, name="r_sb",
                                             tag="renorm")
                            nc.vector.reciprocal(r_sb[:, :], cs[:, :])
                            ls = scrp.tile((1, Nb), F32, name="ls",
                                           tag="renorm")
                            nc.scalar.activation(ls[:, :], cs[:, :], AF.Ln)
                            nc.vector.tensor_tensor(
                                ls[:, :], ls[:, :], mrow[t][:, gs], OP.mult)
                            nc.vector.tensor_tensor(
                                logsel[:, gs], logsel[:, gs], ls[:, :],
                                OP.add)
                            rbc = rbcps.tile((T, Nb), F32, name="rbc")
                            nc.tensor.matmul(
                                rbc[:, :], onesrow[:, :], r_sb[:, :],
                                start=True, stop=True, skip_group_check=True)
                            nc.vector.tensor_tensor(
                                f_slice(tf, g), f_slice(tf, g), rbc[:, :],
                                OP.mult)
                    if t < L:
                        a_cur = apool.tile((T, Bc), F32, name="a_t")
                        for g in range(G):
                            ps = scanps.tile((T, Nb), F32, name="mm_ps")
                            nc.tensor.matmul(
                                ps[:, :], e_mat[:, :],
                                a_prev[:, g * Nb:(g + 1) * Nb],
                                start=True, stop=True, skip_group_check=True)
                            nc.vector.tensor_tensor(
                                a_cur[:, g * Nb:(g + 1) * Nb],
                                ps[:, :], f_slice(t, g), OP.mult)
                        a_prev = a_cur

                # =============== gold path ===============
                nmm = 0
                for b in range(Bc):
                    for ck in range(n_tt):
                        fg = fgp.tile((128, T), F32, name="fg")
                        nc.sync.dma_start(
                            fg[:, :],
                            feats_flat[b:b + 1,
                                       128 * T * ck:128 * T * (ck + 1)]
                            .rearrange("o (p f) -> (o p) f", f=T))
                        tcol = tagsT[ck][:, b:b + 1]
                        ohu = ohp.tile((128, T), F32, name="ohu")
                        nc.vector.tensor_scalar(ohu[:, :], iota48f[:, :],
                                                tcol, None, OP.is_equal)
                        ohms = ohp.tile((128, T), F32, name="ohms")
                        nc.vector.tensor_scalar(
                            ohms[:, :], iota48f[:, :],
                            tagsTsm[ck][:, b:b + 1], None, OP.is_equal)
                        nc.tensor.matmul(c_ps[:, :], ohu[:, :], ohms[:, :],
                                         start=(nmm == 0), stop=False,
                                         skip_group_check=True)
                        nmm += 1
                        scr = scrp.tile((128, T), F32, name="scr", tag="scr")
                        nc.vector.scalar_tensor_tensor(
                            scr[:, :], iota48f[:, :],
                            tagsTm[ck][:, b:b + 1], fg[:, :],
                            OP.is_equal, OP.mult,
                            accum_out=feat_acc[:, b * n_tt + ck:
                                               b * n_tt + ck + 1])
                zrow = cp.tile((1, T), F32)
                nc.vector.memset(zrow[:, :], 0.0)
                nc.tensor.matmul(c_ps[:, :], zrow[:, :], zrow[:, :],
                                 start=False, stop=True,
                                 skip_group_check=True)

                # gold misc terms (b-partition layout)
                featlast = fgp.tile((Bc, T), F32, name="featlast")
                nc.sync.dma_start(featlast[:, :],
                                  feats_flat[:, (L - 1) * T:L * T])
                scrb = scrp.tile((Bc, T), F32, name="scrb", tag="scrb")
                nc.vector.scalar_tensor_tensor(
                    scrb[:, :], iota48f[0:Bc, :], tagsf[:, 0:1],
                    startbc[:, :], OP.is_equal, OP.mult,
                    accum_out=misc_acc[:, 0:1])
                scrb2 = scrp.tile((Bc, T), F32, name="scrb2", tag="scrb")
                mtagl = ohp.tile((Bc, 1), F32, name="mtagl")
                nc.vector.tensor_scalar(mtagl[:, :], maskf[:, L - 1:L],
                                        -100.0, 100.0, OP.mult, OP.add)
                nc.vector.tensor_tensor(mtagl[:, :], mtagl[:, :],
                                        tagsf[:, L - 1:L], OP.add)
                fcor = ohp.tile((Bc, 1), F32, name="fcor")
                nc.vector.scalar_tensor_tensor(
                    scrb2[:, :], iota48f[0:Bc, :], mtagl[:, :],
                    featlast[:, :], OP.is_equal, OP.mult,
                    accum_out=fcor[:, :])
                nc.vector.tensor_scalar(misc_acc[:, 3:4], fcor[:, :], -1.0,
                                        None, OP.mult)
                lenb = cp.tile((Bc, 1), F32)
                nc.vector.tensor_reduce(lenb[:, :], maskf[:, :],
                                        mybir.AxisListType.X, OP.add)
                lm1 = cp.tile((Bc, 1), F32)
                nc.vector.tensor_scalar(lm1[:, :], lenb[:, :], 1.0, None,
                                        OP.subtract)
                scrL = cp.tile((Bc, L), F32)
                lt = cp.tile((Bc, 1), F32)
                nc.vector.scalar_tensor_tensor(
                    scrL[:, :], iotaLf[:, :], lm1[:, :], tagsf[:, :],
                    OP.is_equal, OP.mult, accum_out=lt[:, :])
                scrb3 = scrp.tile((Bc, T), F32, name="scrb3", tag="scrb")
                nc.vector.scalar_tensor_tensor(
                    scrb3[:, :], iota48f[0:Bc, :], lt[:, :], endbc[:, :],
                    OP.is_equal, OP.mult, accum_out=misc_acc[:, 1:2])
                scrb4 = scrp.tile((Bc, T), F32, name="scrb4", tag="scrb")
                fe0 = cp.tile((Bc, 1), F32)
                nc.vector.scalar_tensor_tensor(
                    scrb4[:, :], iota48f[0:Bc, :], lt[:, :], featlast[:, :],
                    OP.is_equal, OP.mult, accum_out=fe0[:, :])
                nc.vector.tensor_tensor(misc_acc[:, 2:3], fe0[:, :],
                                        maskf[:, L - 1:L], OP.mult)

            # =============== end phase ===============
            with (
                tc.tile_pool(name="endp", bufs=2) as ep,
                tc.tile_pool(name="endps", bufs=1, space="PSUM") as epp,
            ):
                gold_ps = epp.tile((1, 1), F32, name="gold_ps")
                scrT = ep.tile((T, T), F32, name="scrT")
                cacc = ep.tile((T, 1), F32, name="cacc")
                nc.vector.tensor_tensor(scrT[:, :], c_ps[:, :],
                                        trans_sb[:, :], OP.mult)
                nc.vector.tensor_reduce(cacc[:, :], scrT[:, :],
                                        mybir.AxisListType.X, OP.add)
                nc.tensor.matmul(gold_ps[:, :], ones128[0:T, :], cacc[:, :],
                                 start=True, stop=False,
                                 skip_group_check=True)
                fred = ep.tile((128, 1), F32, name="fred")
                nc.vector.tensor_reduce(fred[:, :], feat_acc[:, :],
                                        mybir.AxisListType.X, OP.add)
                nc.tensor.matmul(gold_ps[:, :], ones128[:, :], fred[:, :],
                                 start=False, stop=False,
                                 skip_group_check=True)
                mred = ep.tile((Bc, 1), F32, name="mred")
                nc.vector.tensor_reduce(mred[:, :], misc_acc[:, :],
                                        mybir.AxisListType.X, OP.add)
                nc.tensor.matmul(gold_ps[:, :], ones128[0:Bc, :], mred[:, :],
                                 start=False, stop=True,
                                 skip_group_check=True)

                fwd_ps = epp.tile((1, Bc), F32, name="fwd_ps")
                for m in range(n_cap):
                    capt = ep.tile((128, Bc), F32, name="capt", tag="capt")
                    nc.sync.dma_start(
                        capt[:, :],
                        cap_stage[0:1, (8 + 128 * m) * Bc:
                                  (8 + 128 * (m + 1)) * Bc]
                        .rearrange("o (p f) -> o p f", f=Bc))
                    lc = ep.tile((128, Bc), F32, name="lc", tag="lc")
                    nc.scalar.activation(lc[:, :], capt[:, :], AF.Ln)
                    pr = ep.tile((128, Bc), F32, name="pr", tag="pr")
                    nc.vector.tensor_tensor(
                        pr[:, :], lc[:, :], ind[CAP0 // 128 + m][:, :],
                        OP.mult)
                    nc.tensor.matmul(fwd_ps[:, :], ones128[:, :], pr[:, :],
                                     start=(m == 0), stop=(m == n_cap - 1),
                                     skip_group_check=True)
                fwd_sel = ep.tile((1, Bc), F32, name="fwd_sel")
                nc.scalar.copy(fwd_sel[:, :], fwd_ps[:, :])
                lc0 = ep.tile((1, Bc), F32, name="lc0")
                cap0t = ep.tile((1, Bc), F32, name="cap0t")
                nc.sync.dma_start(cap0t[:, :], cap_stage[0:1, 7 * Bc:8 * Bc])
                nc.scalar.activation(lc0[:, :], cap0t[:, :], AF.Ln)
                nc.vector.tensor_tensor(lc0[:, :], lc0[:, :], ind_c0[:, :],
                                        OP.mult)
                nc.vector.tensor_tensor(fwd_sel[:, :], fwd_sel[:, :],
                                        lc0[:, :], OP.add)
                nc.vector.tensor_tensor(fwd_sel[:, :], fwd_sel[:, :],
                                        logsel[:, :], OP.add)
                shifts = ep.tile((1, Bc), F32, name="shifts")
                nc.vector.tensor_scalar(shifts[:, :], lenm1_row[:, :],
                                        A_SHIFT + MU, MU, OP.mult, OP.add)
                nc.vector.tensor_tensor(fwd_sel[:, :], fwd_sel[:, :],
                                        shifts[:, :], OP.add)
                fwd_tot = ep.tile((1, 1), F32, name="fwd_tot")
                nc.vector.tensor_reduce(fwd_tot[:, :], fwd_sel[:, :],
                                        mybir.AxisListType.X, OP.add)
                loss = ep.tile((1, 1), F32, name="loss")
                nc.vector.tensor_tensor(loss[:, :], fwd_tot[:, :],
                                        gold_ps[:, :], OP.subtract)
                nc.sync.dma_start(out_d.ap(), loss[:, :])
                if dbg:
                    gsb = ep.tile((1, 1), F32, name="gsb")
                    nc.scalar.copy(gsb[:, :], gold_ps[:, :])
                    fsel0 = ep.tile((1, Bc), F32, name="fsel0")
                    nc.scalar.copy(fsel0[:, :], fwd_ps[:, :])
                    nc.sync.dma_start(dbg_d.ap()[0:1, :], logsel[:, :])
                    nc.sync.dma_start(dbg_d.ap()[1:2, :], fwd_sel[:, :])
                    nc.sync.dma_start(dbg_d.ap()[2:3, :], lenm1_row[:, :])
                    nc.sync.dma_start(dbg_d.ap()[3:4, :], lc0[:, :])
                    nc.sync.dma_start(dbg_d.ap()[4:5, :], fsel0[:, :])
                    nc.sync.dma_start(dbg_d.ap()[5:6, 0:1], gsb[:, :])

    nc.compile()
    return nc


def shard_inputs(feats, transitions, start_transitions, end_transitions,
                 tags, mask, n_cores=N_CORES):
    feats = np.ascontiguousarray(np.asarray(feats, dtype=np.float32))
    transitions = np.ascontiguousarray(
        np.asarray(transitions, dtype=np.float32))
    start_transitions = np.ascontiguousarray(
        np.asarray(start_transitions, dtype=np.float32))
    end_transitions = np.ascontiguousarray(
        np.asarray(end_transitions, dtype=np.float32))
    tags = np.ascontiguousarray(np.asarray(tags).astype(np.int32))
    mask = np.ascontiguousarray(np.asarray(mask).astype(np.int32))
    Bc = feats.shape[0] // n_cores
    in_maps = []
    for c in range(n_cores):
        s = slice(c * Bc, (c + 1) * Bc)
        in_maps.append({
            "feats": feats[s],
            "trans": transitions,
            "start": start_transitions,
            "end": end_transitions,
            "tags": tags[s],
            "mask": mask[s],
        })
    return in_maps, feats.shape


def kernel(feats, transitions, start_transitions, end_transitions, tags,
           mask, **_ignored):
    in_maps, (Bf, L, _) = shard_inputs(
        feats, transitions, start_transitions, end_transitions, tags, mask)
    nc = build_program(L=L, Bc=Bf // N_CORES)
    res = run_bass_kernel_spmd(nc, in_maps, core_ids=list(range(N_CORES)))
    total = sum(float(r["out"][0, 0]) for r in res.results)
    return np.float32(total)



# revision 40
# speedup vs baseline: 1.8350x; 1.8350x over previous
"""Trainium2 Bass kernel for CRF negative-log-likelihood loss.

nn_CRF (B=512, L=1024, T=48), data-parallel over 8 NeuronCores (Bc=64
rows per core); host sums the 8 scalar partials.

Design (v2, segmented scan):
  Forward (partition function): the linear-domain scan
  A_t = (E^T A_{t-1}) * F_t (E = exp(trans - log T), F = exp(feat - MU))
  is split into NSEG=24 time segments of SEGLEN=43 steps with W=12
  warmup steps each (Hilbert-metric contraction of E makes the
  direction forget its init in ~8 steps, and diagonal F scalings are
  Hilbert isometries, so a warm-started segment converges to the true
  forward direction up to a per-column scale).  No renormalisation is
  needed inside a segment: fp32/bf16 exponent range absorbs the drift,
  and per-segment log-gains telescope through boundary column sums.
  Segments are packed 2-per-partition-group x 6-per-column-group into
  C=2 independent chains of (112, 384) matmul+multiply steps, with an
  exact side chain (48, 64) covering t in [0, 56) to anchor the
  telescoped magnitude.  The stationary matrix carries extra columns
  that compute end-capture rows and column sums for free; those rows
  ride through the F-multiply (F rows 48-63 are 1.0 via natfb padding)
  and are staged to DRAM, reloaded in (slot, seg*batch) layout, and
  selected by per-row length indicators.
  F tiles are produced by XBAR DMA transposes (128-source-column tiles)
  of a pre-exponentiated, 64-element-padded bf16 copy of feats (natfb),
  so the PE does no transposes and the ACT does no PSUM evacuation.
  Gold (numerator): one-hot tiles (bf16 tensor_scalar is_equal) over
  127-step chunks; bigram counts via offset-partition matmuls
  C += ohu[0:127]^T @ ohu[1:128] (mask baked into tags, so the pair
  weight mask_t*mask_{t+1} = mask_{t+1} is automatic); feature gathers
  via fused scalar_tensor_tensor ops split across DVE and Pool.
"""

import math

import numpy as np

import concourse.bacc as bacc
import concourse.mybir as mybir
import concourse.tile as tile
from concourse.bass import AP
from concourse.bass_utils import run_bass_kernel_spmd

F32 = mybir.dt.float32
BF16 = mybir.dt.bfloat16
I32 = mybir.dt.int32
AF = mybir.ActivationFunctionType
OP = mybir.AluOpType

B_FULL = 512
N_CORES = 8
BC = B_FULL // N_CORES  # 64
L_FULL = 1024
T = 48

MU = 0.51
ALPHA = math.log(T)
SEGLEN = 43
NSEG = 24          # segments s = 1..24, seg s main range [43s, 43s+43)
W = 12             # warmup steps
SLOTS = 56         # tau = 0..55; mm steps tau = 1..56
NTB = 576          # natfb window width in t-slots
W0 = 0             # chain-0 window start (t)
W1 = 508           # chain-1 window start (t)
NCH = 6            # column groups (m) per chain
WCH = NCH * BC     # 384 = chain column width
GOLD_CK = 8        # 128-step gold chunks


def build_program(dbg=False):
    L = L_FULL
    Bc = BC
    nc = bacc.Bacc("TRN2", target_bir_lowering=False, debug=False)

    feats_d = nc.dram_tensor("feats", (Bc, L, T), F32, kind="ExternalInput")
    trans_d = nc.dram_tensor("trans", (T, T), F32, kind="ExternalInput")
    start_d = nc.dram_tensor("start", (T,), F32, kind="ExternalInput")
    end_d = nc.dram_tensor("end", (T,), F32, kind="ExternalInput")
    tags_d = nc.dram_tensor("tags", (Bc, L), I32, kind="ExternalInput")
    mask_d = nc.dram_tensor("mask", (Bc, L), I32, kind="ExternalInput")
    out_d = nc.dram_tensor("out", (1, 1), F32, kind="ExternalOutput")
    dbg_d = (nc.dram_tensor("dbg", (8, Bc), F32, kind="ExternalOutput")
             if dbg else None)

    feats_flat = feats_d.ap().rearrange("b l t -> b (l t)")

    with tile.TileContext(nc) as tc:
        with (
            tc.tile_pool(name="const", bufs=1) as cp,
            tc.tile_pool(name="cps", bufs=1, space="PSUM") as cpp,
            tc.tile_pool(name="dramp", bufs=1, space="DRAM") as dp,
        ):
            # ---------------- constants ----------------
            iota48i = cp.tile((128, T), I32)
            nc.gpsimd.iota(iota48i[:, :], [[1, T]], channel_multiplier=0)
            iota48f = cp.tile((128, T), F32)
            nc.vector.tensor_copy(iota48f[:, :], iota48i[:, :])
            iota48b = cp.tile((128, T), BF16)
            nc.vector.tensor_copy(iota48b[:, :], iota48i[:, :])

            iota64i = cp.tile((64, 64), I32)
            nc.gpsimd.iota(iota64i[:, :], [[1, 64]], channel_multiplier=0)
            iotaPi = cp.tile((64, 1), I32)
            nc.gpsimd.iota(iotaPi[:, :], [[1, 1]], channel_multiplier=1)
            iota64f = cp.tile((64, 64), F32)
            nc.vector.tensor_copy(iota64f[:, :], iota64i[:, :])
            iotaPf = cp.tile((64, 1), F32)
            nc.vector.tensor_copy(iotaPf[:, :], iotaPi[:, :])
            identMf = cp.tile((64, 64), F32)
            nc.vector.tensor_scalar(
                identMf[:, :], iota64f[:, :], iotaPf[:, :], None, OP.is_equal)
            identMb = cp.tile((64, 64), BF16)
            nc.vector.tensor_copy(identMb[:, :], identMf[:, :])

            iotaLf = cp.tile((Bc, L), F32)

            ones128f = cp.tile((128, 1), F32)
            nc.vector.memset(ones128f[:, :], 1.0)
            ones128b = cp.tile((128, 1), BF16)
            nc.vector.memset(ones128b[:, :], 1.0)

            bias_mu = cp.tile((128, 1), F32)
            nc.vector.memset(bias_mu[:, :], -MU)
            bias_a = cp.tile((T, 1), F32)
            nc.vector.memset(bias_a[:, :], -ALPHA)

            # ---------------- params ----------------
            trans_sb = cp.tile((T, T), F32)
            nc.sync.dma_start(trans_sb[:, :], trans_d.ap())
            e_f32 = cp.tile((T, T), F32)
            nc.scalar.activation(e_f32[:, :], trans_sb[:, :], AF.Exp,
                                 bias=bias_a[:, :])
            e_b = cp.tile((T, T), BF16)
            nc.vector.tensor_copy(e_b[:, :], e_f32[:, :])

            end_sb = cp.tile((T, 1), F32)
            nc.sync.dma_start(end_sb[:, :], end_d.ap().unsqueeze(1))
            expend_f = cp.tile((T, 1), F32)
            nc.scalar.activation(expend_f[:, :], end_sb[:, :], AF.Exp)
            expend_b = cp.tile((T, 1), BF16)
            nc.vector.tensor_copy(expend_b[:, :], expend_f[:, :])

            start_sb = cp.tile((T, 1), F32)
            nc.sync.dma_start(start_sb[:, :], start_d.ap().unsqueeze(1))
            expstart = cp.tile((T, 1), F32)
            nc.scalar.activation(expstart[:, :], start_sb[:, :], AF.Exp)

            startbc = cp.tile((Bc, T), F32)
            nc.sync.dma_start(
                startbc[:, :], start_d.ap().unsqueeze(0).partition_broadcast(Bc))
            endbc = cp.tile((Bc, T), F32)
            nc.sync.dma_start(
                endbc[:, :], end_d.ap().unsqueeze(0).partition_broadcast(Bc))

            # main stationary (112, 112):
            #  rows 0-47 (block A), rows 64-111 (block B), rows 48-63 zero
            #  cols 0-47 = E(A), 64-111 = E(B), 48 = capA, 49 = capB,
            #  50 = sumA, 51 = sumB, 52-63 zero
            s_main = cp.tile((112, 112), BF16)
            nc.vector.memset(s_main[:, :], 0.0)
            nc.vector.tensor_copy(s_main[0:T, 0:T], e_b[:, :])
            nc.sync.dma_start(s_main[64:112, 64:112], e_b[:, :])
            nc.vector.tensor_copy(s_main[0:T, 48:49], expend_b[:, :])
            nc.sync.dma_start(s_main[64:112, 49:50], expend_b[:, :])
            nc.vector.memset(s_main[0:T, 50:51], 1.0)
            nc.vector.memset(s_main[64:112, 51:52], 1.0)

            # side stationary (48, 50): cols 0-47 E, 48 = cap, 49 = sum
            s_side = cp.tile((T, 50), BF16)
            nc.vector.tensor_copy(s_side[:, 0:T], e_b[:, :])
            nc.vector.tensor_copy(s_side[:, 48:49], expend_b[:, :])
            nc.vector.memset(s_side[:, 49:50], 1.0)

            # ---------------- tags / mask prep ----------------
            prep_scope = tc.tile_pool(name="prepsb", bufs=1)
            prp = prep_scope.__enter__()
            iotaLi = prp.tile((Bc, L), I32)
            nc.gpsimd.iota(iotaLi[:, :], [[1, L]], channel_multiplier=0)
            nc.vector.tensor_copy(iotaLf[:, :], iotaLi[:, :])
            tags_i = prp.tile((Bc, L), I32)
            nc.sync.dma_start(tags_i[:, :], tags_d.ap())
            tagsf = cp.tile((Bc, L), F32)
            nc.vector.tensor_copy(tagsf[:, :], tags_i[:, :])
            mask_i = prp.tile((Bc, L), I32)
            nc.sync.dma_start(mask_i[:, :], mask_d.ap())
            maskf = cp.tile((Bc, L), F32)
            nc.vector.tensor_copy(maskf[:, :], mask_i[:, :])
            tagsmb = prp.tile((Bc, L), BF16)
            moff = prp.tile((Bc, L), F32)
            nc.vector.tensor_scalar(moff[:, :], maskf[:, :], -100.0, 100.0,
                                    OP.mult, OP.add)
            tagsm_f = prp.tile((Bc, L), F32)
            nc.vector.tensor_tensor(tagsm_f[:, :], tagsf[:, :], moff[:, :],
                                    OP.add)
            nc.vector.tensor_copy(tagsmb[:, :], tagsm_f[:, :])

            tagsmSb = prp.tile((Bc, L), BF16)
            nc.vector.memset(tagsmSb[:, :], 100.0)
            nc.vector.tensor_copy(tagsmSb[:, 0:L - 1], tagsm_f[:, 1:L])

            lenb = cp.tile((Bc, 1), F32)
            nc.vector.tensor_reduce(lenb[:, :], maskf[:, :],
                                    mybir.AxisListType.X, OP.add)

            # transposed masked tags: 9 tiles (128, 64), 127-stride chunks
            tagsTm = []
            tagsTmS = []
            with tc.tile_pool(name="prepps", bufs=2, space="PSUM") as ppp:
                for k in range(GOLD_CK):
                    ps = ppp.tile((128, Bc), BF16, name=f"tps_{k}", tag="tps",
                                  bufs=2)
                    nc.tensor.transpose(ps[:, :],
                                        tagsmb[:, 128 * k:128 * (k + 1)],
                                        identMb[:, :])
                    tt = cp.tile((128, Bc), BF16, name=f"tagsTm_{k}")
                    nc.vector.tensor_copy(tt[:, :], ps[:, :])
                    tagsTm.append(tt)
                    ps2 = ppp.tile((128, Bc), BF16, name=f"tps2_{k}",
                                   tag="tps", bufs=2)
                    nc.tensor.transpose(ps2[:, :],
                                        tagsmSb[:, 128 * k:128 * (k + 1)],
                                        identMb[:, :])
                    tt2 = cp.tile((128, Bc), BF16, name=f"tagsTmS_{k}")
                    nc.vector.tensor_copy(tt2[:, :], ps2[:, :])
                    tagsTmS.append(tt2)
                # len row (1, 64) via transpose
                lps = ppp.tile((1, Bc), F32, name="lps", tag="lps", bufs=1)
                nc.tensor.transpose(lps[:, :], lenb[:, :], identMf[:, :])
                lenrow = cp.tile((1, Bc), F32)
                nc.vector.tensor_copy(lenrow[:, :], lps[:, :])
            prep_scope.__exit__(None, None, None)

            # (natfb/natfbS/A tiles are allocated inside the scan scope
            # below so their SBUF frees before the end phase)
            # ---------------- natfb: padded exp'd bf16 feats ----------------
            # (128, 56*12*64): row c*64+b holds chain c; column layout
            # (tau*12 + strip)*64 + jj with strip = m*2 + tp, so each
            # XBAR transpose slab input is CONTIGUOUS and 128-element
            # source groups give partitions tp*64 + jj.  jj 48-63 = 1.0
            # (become the F=1 ride-through rows after transpose).
            bigp_scope = tc.tile_pool(name="bigp", bufs=1)
            bigp = bigp_scope.__enter__()
            natfb = bigp.tile((128, SLOTS * 12 * 64), BF16)
            nc.vector.memset(
                natfb[:, :].rearrange("p (ts jj) -> p ts jj", ts=SLOTS * 12,
                                      jj=64)[:, :, T:64], 1.0)
            # invalid tails of the two clipped strips (chain 1, s=23, 24)
            # strip (tp=1, m=4) -> strip idx 9: slots tau >= 47 invalid
            # strip (tp=1, m=5) -> strip idx 11: slots tau >= 4 invalid
            nc.vector.memset(
                natfb[64:128, :].rearrange(
                    "p (t s jj) -> p t s jj", t=SLOTS, s=12,
                    jj=64)[:, 47:SLOTS, 9, 0:T], 1.0)
            nc.vector.memset(
                natfb[64:128, :].rearrange(
                    "p (t s jj) -> p t s jj", t=SLOTS, s=12,
                    jj=64)[:, 4:SLOTS, 11, 0:T], 1.0)
            # side-chain feats: plain t-slot layout, t in [0, 56)
            natfbS = bigp.tile((Bc, SLOTS * 64), BF16)
            nc.vector.memset(
                natfbS[:, :].rearrange("p (t jj) -> p t jj", t=SLOTS,
                                       jj=64)[:, :, T:64], 1.0)

            # stage DRAM: per chain (4, 57*384) bf16
            stage_dr = [dp.tile((4, 57 * WCH), BF16, name=f"stage_{c}")
                        for c in range(2)]

            with (
                tc.tile_pool(name="chkp", bufs=2) as chp,
                tc.tile_pool(name="fslab", bufs=2) as fsp,
                tc.tile_pool(name="fside", bufs=2) as fsdp,
                tc.tile_pool(name="scanps", bufs=3, space="PSUM") as sps,
                tc.tile_pool(name="scanps2", bufs=3, space="PSUM") as sps2,
                tc.tile_pool(name="sideps", bufs=1, space="PSUM") as sdps,
                tc.tile_pool(name="ohp", bufs=4) as ohp,
                tc.tile_pool(name="bounce", bufs=1) as bpp,
                tc.tile_pool(name="fgp", bufs=2) as fgp,
                tc.tile_pool(name="scrp", bufs=2) as scrp,
            ):
                # ---- feats strip DMAs + exp into natfb ----
                natfb4 = natfb[:, :].rearrange(
                    "p (t s jj) -> p t s jj", t=SLOTS, s=12, jj=64)
                natfb4c1 = natfb[64:128, :].rearrange(
                    "p (t s jj) -> p t s jj", t=SLOTS, s=12, jj=64)

                HS = SLOTS // 2  # 28-slot half strips

                def emit_strip(tp, m):
                    # strip = m*2 + tp; seg s_c = 12c + 6tp + m + 1;
                    # t range [43s - 12, 43s + 44)
                    strip = m * 2 + tp
                    s0 = 6 * tp + m + 1
                    s1 = s0 + 12
                    nvalid1 = min(SLOTS, max(0, L - (SEGLEN * s1 - 12)))
                    for h in range(2):
                        t0 = SEGLEN * s0 - 12 + h * HS
                        tsl = slice(h * HS, (h + 1) * HS)
                        if nvalid1 == SLOTS:
                            ch = chp.tile((128, HS * T), F32, name="natf")
                            in_ap = AP(feats_flat.tensor, t0 * T,
                                       [[516 * T, 2], [L * T, Bc],
                                        [T, HS], [1, T]])
                            nc.sync.dma_start(ch[:, :], in_ap)
                            nc.scalar.activation(
                                natfb4[:, tsl, strip, 0:T],
                                ch[:, :], AF.Exp, bias=bias_mu[:, :])
                        else:
                            ch = chp.tile((128, HS * T), F32, name="natf")
                            in_ap = AP(feats_flat.tensor, t0 * T,
                                       [[L * T, Bc], [T, HS], [1, T]])
                            nc.sync.dma_start(ch[0:Bc, :], in_ap)
                            nc.scalar.activation(
                                natfb4[0:Bc, tsl, strip, 0:T],
                                ch[0:Bc, :], AF.Exp, bias=bias_mu[0:Bc, :])
                            nv = min(max(nvalid1 - h * HS, 0), HS)
                            if nv > 0:
                                ch2 = chp.tile((128, HS * T), F32,
                                               name="natf")
                                in2 = AP(feats_flat.tensor,
                                         (SEGLEN * s1 - 12 + h * HS) * T,
                                         [[L * T, Bc], [T, nv], [1, T]])
                                nc.sync.dma_start(ch2[0:Bc, 0:nv * T], in2)
                                nc.scalar.activation(
                                    natfb4c1[:, h * HS:h * HS + nv,
                                             strip, 0:T],
                                    ch2[0:Bc, 0:nv * T], AF.Exp,
                                    bias=bias_mu[0:Bc, :])

                # side strip first (unblocks the side chain)
                for h in range(2):
                    HSs = SLOTS // 2
                    chS = chp.tile((128, HSs * T), F32, name="natf")
                    nc.sync.dma_start(
                        chS[0:Bc, :],
                        AP(feats_flat.tensor, h * HSs * T,
                           [[L * T, Bc], [T, HSs], [1, T]]))
                    nc.scalar.activation(
                        natfbS[:, :].rearrange(
                            "p (t jj) -> p t jj", t=SLOTS,
                            jj=64)[:, h * HSs:(h + 1) * HSs, 0:T],
                        chS[0:Bc, :], AF.Exp, bias=bias_mu[0:Bc, :])
                for m in range(NCH):
                    for tp in range(2):
                        emit_strip(tp, m)

                # ---- side chain (exact, t in [0, 56]) ----
                # F side slabs: q covers tau in [8q, 8q+8)
                side_slabs = {}

                natfbS_t = natfbS[:, :].tensor

                def emit_side_slab(q):
                    sl = fsdp.tile((128, 4 * 64), BF16, name="fside")
                    in_ap = AP(natfbS_t, 8 * q * 64,
                               [[SLOTS * 64, Bc], [1, 512]])
                    nc.scalar.dma_start_transpose(
                        sl[:, :].rearrange("p (e b) -> p e b", e=4, b=64),
                        in_ap)
                    side_slabs[q] = sl

                def side_f(tau):
                    sl = side_slabs[tau // 8]
                    p0 = (tau % 2) * 64
                    c0 = ((tau // 2) % 4) * 64
                    return sl[p0:p0 + 50, c0:c0 + 64]

                emit_side_slab(0)
                emit_side_slab(1)

                side_pool = tc.tile_pool(name="sidea", bufs=3)
                sap = side_pool.__enter__()
                a_side = sap.tile((50, Bc), BF16, name="a_side")
                # A_side(0) = exp(start) * F_0  (rows 48-49 will be junk)
                nc.vector.memset(a_side[32:50, :], 1.0)
                nc.vector.tensor_scalar(a_side[0:T, :],
                                        side_slabs[0][0:T, 0:64],
                                        expstart[:, :], None, OP.mult)
                lnsideS43 = cp.tile((1, Bc), F32)
                sums_side44 = cp.tile((1, Bc), BF16)

                for tau in range(1, 45):
                    if tau % 8 == 1 and tau // 8 + 2 <= 5:
                        emit_side_slab(tau // 8 + 2)
                    ps = sdps.tile((50, Bc), F32, name="side_ps")
                    nc.tensor.matmul(ps[:, :], s_side[:, :], a_side[0:T, :],
                                     start=True, stop=True,
                                     skip_group_check=True)
                    a_new = sap.tile((50, Bc), BF16, name="a_side")
                    nc.vector.tensor_tensor(a_new[:, :], ps[:, :],
                                            side_f(tau), OP.mult)
                    if tau == 44:
                        nc.sync.dma_start(sums_side44[:, :],
                                          a_new[49:50, :])
                    a_side = a_new
                nc.scalar.activation(lnsideS43[:, :], sums_side44[:, :],
                                     AF.Ln)
                side_pool.__exit__(None, None, None)

                # ---- main F slabs ----
                main_slabs = {}
                natfb_t = natfb[:, :].tensor

                def emit_main_slab(c, q):
                    sl = fsp.tile((128, 8 * WCH), BF16, name="fslab")
                    if c == 0:
                        in_ap = AP(natfb_t, q * 8 * 768,
                                   [[SLOTS * 12 * 64, Bc], [1, 8 * 768]])
                    else:
                        # XBAR input must start at partition 0: bounce
                        # the chain-1 span down via an SBUF DMA first
                        bt = bpp.tile((Bc, 8 * 768), BF16, name="bounce")
                        nc.sync.dma_start(
                            bt[:, :],
                            AP(natfb_t,
                               64 * (SLOTS * 12 * 64) + q * 8 * 768,
                               [[SLOTS * 12 * 64, Bc], [1, 8 * 768]]))
                        in_ap = bt[:, :]
                    nc.scalar.dma_start_transpose(
                        sl[:, :].rearrange("p (e b) -> p e b", e=8 * NCH,
                                           b=64),
                        in_ap)
                    main_slabs[(c, q)] = sl

                for c in range(2):
                    emit_main_slab(c, 0)
                    emit_main_slab(c, 1)

                # ---- A ping-pong tiles ----
                app = [[bigp.tile((112, 8 * WCH), BF16, name=f"A_{c}_{i}")
                        for i in range(2)] for c in range(2)]
                for c in range(2):
                    t0 = app[c][0]
                    nc.vector.memset(t0[0:64, 0:WCH], 0.0)
                    nc.vector.memset(t0[0:52, 0:WCH], 1.0)
                    nc.vector.memset(t0[64:112, 0:WCH], 1.0)

                # ---- gold work generator (interleaved) ----
                c_ps = cpp.tile((T, T), F32, name="c_ps")
                feat_acc = cp.tile((128, 64), F32)
                nc.vector.memset(feat_acc[:, :], 0.0)
                # b-major iota: val[p, b*48+j] = j
                iota384i = cp.tile((128, 384), I32)
                nc.gpsimd.iota(iota384i[:, :], [[0, 8], [1, T]],
                               channel_multiplier=0)
                iota384b = cp.tile((128, 384), BF16)
                nc.vector.tensor_copy(iota384b[:, :], iota384i[:, :])

                gold_units = [(o, k) for o in range(8)
                              for k in range(GOLD_CK)]
                n_units = len(gold_units)
                gold_pos = [0]
                first_c = [True]

                def emit_gold(n):
                    for _ in range(n):
                        u = gold_pos[0]
                        if u >= n_units:
                            return
                        o, k = gold_units[u]
                        fg8 = fgp.tile((128, 384), F32, name="fg8")
                        in_ap = AP(feats_flat.tensor,
                                   8 * o * L * T + 128 * k * T,
                                   [[T, 128], [L * T, 8], [1, T]])
                        nc.sync.dma_start(fg8[:, :], in_ap)
                        ohu8 = ohp.tile((128, 384), BF16, name="ohu8")
                        tu = tagsTm[k][:, :].tensor
                        nc.vector.tensor_tensor(
                            ohu8[:, :], iota384b[:, :],
                            AP(tu, 8 * o, [[Bc, 128], [1, 8], [0, T]]),
                            OP.is_equal)
                        ohs8 = ohp.tile((128, 384), BF16, name="ohs8")
                        ts_ = tagsTmS[k][:, :].tensor
                        nc.vector.tensor_tensor(
                            ohs8[:, :], iota384b[:, :],
                            AP(ts_, 8 * o, [[Bc, 128], [1, 8], [0, T]]),
                            OP.is_equal)
                        for b in range(8):
                            nc.tensor.matmul(
                                c_ps[:, :],
                                ohu8[:, b * T:(b + 1) * T],
                                ohs8[:, b * T:(b + 1) * T],
                                start=first_c[0], stop=False,
                                skip_group_check=True)
                            first_c[0] = False
                        scr = scrp.tile((128, 384), F32, name="scr",
                                        tag="scr")
                        nc.vector.scalar_tensor_tensor(
                            scr[:, :], ohu8[:, :], 1.0, fg8[:, :],
                            OP.mult, OP.mult,
                            accum_out=feat_acc[:, u:u + 1])
                        gold_pos[0] += 1

                # ---- main scan ----
                def a_slice(c, tau):
                    return app[c][(tau // 8) % 2][:, (tau % 8) * WCH:
                                                  (tau % 8 + 1) * WCH]

                def f_slice(c, tau):
                    # step 56 only needs the F=1 ride-through rows; reuse
                    # slot 55 (A rows 0-111 of state 56 are never used)
                    tau = min(tau, SLOTS - 1)
                    sl = main_slabs[(c, tau // 8)]
                    return sl[0:112, (tau % 8) * WCH:(tau % 8 + 1) * WCH]

                emit_gold(6)
                for tau in range(1, SLOTS + 1):
                    if tau % 8 == 1:
                        q = tau // 8 + 2
                        if q <= 6:
                            for c in range(2):
                                emit_main_slab(c, q)
                    for c in range(2):
                        pool = sps if c == 0 else sps2
                        ps = pool.tile((112, WCH), F32, name=f"mm_{c}")
                        nc.tensor.matmul(ps[:, :], s_main[:, :],
                                         a_slice(c, tau - 1)[0:112, :],
                                         start=True, stop=True,
                                         skip_group_check=True)
                        nc.vector.tensor_tensor(a_slice(c, tau), ps[:, :],
                                                f_slice(c, tau), OP.mult)
                    if tau % 8 == 7 or tau == SLOTS:
                        pass
                    if tau % 8 == 0:
                        q = tau // 8 - 1
                        for c in range(2):
                            nc.sync.dma_start(
                                stage_dr[c][0:4,
                                            q * 8 * WCH:(q + 1) * 8 * WCH],
                                app[c][q % 2][48:52, :])
                    emit_gold(2)
                # final partial block: slot 56 = A(56)
                for c in range(2):
                    nc.sync.dma_start(
                        stage_dr[c][0:4, 56 * WCH:57 * WCH],
                        app[c][1][48:52, 0:WCH])
                emit_gold(n_units)
                zrow = cp.tile((1, T), BF16)
                nc.vector.memset(zrow[:, :], 0.0)
                nc.tensor.matmul(c_ps[:, :], zrow[:, :], zrow[:, :],
                                 start=False, stop=True,
                                 skip_group_check=True)

                # ---- gold misc terms (start, end-transitions) ----
                misc_acc = cp.tile((Bc, 2), F32)
                scrb = scrp.tile((Bc, T), F32, name="scrb", tag="scrb")
                nc.vector.scalar_tensor_tensor(
                    scrb[:, :], iota48f[0:Bc, :], tagsf[:, 0:1],
                    startbc[:, :], OP.is_equal, OP.mult,
                    accum_out=misc_acc[:, 0:1])
                lm1 = cp.tile((Bc, 1), F32)
                nc.vector.tensor_scalar(lm1[:, :], lenb[:, :], 1.0, None,
                                        OP.subtract)
                scrL = cp.tile((Bc, L), F32)
                lt = cp.tile((Bc, 1), F32)
                nc.vector.scalar_tensor_tensor(
                    scrL[:, :], iotaLf[:, :], lm1[:, :], tagsf[:, :],
                    OP.is_equal, OP.mult, accum_out=lt[:, :])
                scrb3 = scrp.tile((Bc, T), F32, name="scrb3", tag="scrb")
                nc.vector.scalar_tensor_tensor(
                    scrb3[:, :], iota48f[0:Bc, :], lt[:, :], endbc[:, :],
                    OP.is_equal, OP.mult, accum_out=misc_acc[:, 1:2])

            bigp_scope.__exit__(None, None, None)

            # =============== end phase ===============
            with (
                tc.tile_pool(name="endp", bufs=1) as ep,
                tc.tile_pool(name="endps", bufs=1, space="PSUM") as epp,
                tc.tile_pool(name="endps2", bufs=2, space="PSUM") as epp2,
            ):
                # gold assembly
                gold_ps = epp.tile((1, 1), F32, name="gold_ps")
                scrT = ep.tile((T, T), F32, name="scrT")
                cacc = ep.tile((T, 1), F32, name="cacc")
                nc.vector.tensor_tensor(scrT[:, :], c_ps[:, :],
                                        trans_sb[:, :], OP.mult)
                nc.vector.tensor_reduce(cacc[:, :], scrT[:, :],
                                        mybir.AxisListType.X, OP.add)
                nc.tensor.matmul(gold_ps[:, :], ones128f[0:T, :], cacc[:, :],
                                 start=True, stop=False,
                                 skip_group_check=True)
                fred = ep.tile((128, 1), F32, name="fred")
                nc.vector.tensor_reduce(fred[:, :], feat_acc[:, :],
                                        mybir.AxisListType.X, OP.add)
                nc.tensor.matmul(gold_ps[:, :], ones128f[:, :],
                                 fred[:, :], start=False, stop=False,
                                 skip_group_check=True)
                mred = ep.tile((Bc, 1), F32, name="mred")
                nc.vector.tensor_reduce(mred[:, :], misc_acc[:, :],
                                        mybir.AxisListType.X, OP.add)
                nc.tensor.matmul(gold_ps[:, :], ones128f[0:Bc, :],
                                 mred[:, :], start=False, stop=True,
                                 skip_group_check=True)

                # reload caps/sums: (57, 768) per chain
                capsre, sumsre, capsLn, sumsLn = [], [], [], []
                for c in range(2):
                    cr = ep.tile((57, 2 * WCH), BF16, name=f"capsre_{c}")
                    sr = ep.tile((57, 2 * WCH), BF16, name=f"sumsre_{c}")
                    st_t = stage_dr[c][:, :].tensor
                    nc.sync.dma_start(
                        cr[:, :], AP(st_t, 0,
                                     [[WCH, 57], [57 * WCH, 2],
                                      [64, NCH], [1, 64]]))
                    nc.sync.dma_start(
                        sr[:, :], AP(st_t, 2 * 57 * WCH,
                                     [[WCH, 57], [57 * WCH, 2],
                                      [64, NCH], [1, 64]]))
                    cl = ep.tile((57, 2 * WCH), F32, name=f"capsLn_{c}")
                    sl_ = ep.tile((57, 2 * WCH), F32, name=f"sumsLn_{c}")
                    nc.scalar.activation(cl[:, :], cr[:, :], AF.Ln)
                    nc.scalar.activation(sl_[:, :], sr[:, :], AF.Ln)
                    capsre.append(cr)
                    sumsre.append(sr)
                    capsLn.append(cl)
                    sumsLn.append(sl_)

                # lenrep (1, 1536) f32
                lenrep = ep.tile((1, 24 * 64), F32, name="lenrep")
                nc.vector.tensor_copy(lenrep[:, 0:64], lenrow[:, :])
                for w_ in (64, 128, 256, 512):
                    nc.vector.tensor_copy(lenrep[:, w_:2 * w_],
                                          lenrep[:, 0:w_])
                nc.vector.tensor_copy(lenrep[:, 1024:1536],
                                      lenrep[:, 0:512])

                # per-chain sigma indicator + capture select
                ones_row = ep.tile((1, 64), F32, name="ones_row")
                nc.vector.memset(ones_row[:, :], 1.0)
                comb = ep.tile((1, 24 * 64), F32, name="comb")
                for c in range(2):
                    io = ep.tile((57, 2 * WCH), I32, name=f"indio_{c}")
                    nc.gpsimd.iota(io[:, :], [[258, 2], [43, NCH], [0, 64]],
                                   channel_multiplier=1)
                    iof = ep.tile((57, 2 * WCH), F32,
                                  name=f"indiof_{c}")
                    nc.vector.tensor_copy(iof[:, :], io[:, :])
                    nc.vector.memset(iof[0:13, :], -9999.0)
                    if c == 0:
                        negrow = ep.tile((1, 2 * WCH), F32, name="negrow")
                        nc.vector.memset(negrow[:, :], -9999.0)
                    nc.sync.dma_start(iof[56:57, :], negrow[:, :])
                    lr_c = ep.tile((1, 2 * WCH), F32, name=f"lrc_{c}")
                    nc.vector.tensor_scalar(
                        lr_c[:, :], lenrep[:, 0:2 * WCH],
                        float(31 + 516 * c), None, OP.subtract)
                    for h in range(2):
                        hs = slice(h * WCH, (h + 1) * WCH)
                        lps_c = epp2.tile((57, WCH), F32,
                                          name=f"lps_{c}_{h}", tag="lps",
                                          bufs=2)
                        nc.tensor.matmul(lps_c[:, :], ones_row[0:1, 0:57],
                                         lr_c[:, hs], start=True, stop=True,
                                         skip_group_check=True)
                        ind = ep.tile((57, WCH), F32,
                                      name=f"ind_{c}_{h}")
                        nc.vector.tensor_tensor(ind[:, :], iof[:, hs],
                                                lps_c[:, :], OP.is_equal)
                        pr = ep.tile((57, WCH), F32,
                                      name=f"pr_{c}_{h}")
                        nc.vector.tensor_tensor(pr[:, :], capsLn[c][:, hs],
                                                ind[:, :], OP.mult)
                        fsel = epp2.tile((1, WCH), F32,
                                         name=f"fsel_{c}_{h}", tag="fsel",
                                         bufs=2)
                        nc.tensor.matmul(fsel[:, :], ones128f[0:57, :],
                                         pr[:, :], start=True, stop=True,
                                         skip_group_check=True)
                        nc.vector.tensor_copy(
                            comb[:, c * 768 + h * WCH:
                                 c * 768 + (h + 1) * WCH], fsel[:, :])
                Gall = ep.tile((1, 24 * 64), F32, name="Gall")
                s13 = ep.tile((1, 24 * 64), F32, name="s13")
                s56 = ep.tile((1, 24 * 64), F32, name="s56")
                for c in range(2):
                    nc.sync.dma_start(s13[:, c * 768:(c + 1) * 768],
                                      sumsLn[c][13:14, :])
                    nc.sync.dma_start(s56[:, c * 768:(c + 1) * 768],
                                      sumsLn[c][56:57, :])
                nc.vector.tensor_tensor(Gall[:, :], s56[:, :], s13[:, :],
                                        OP.subtract)
                # ge[s] = (len-1 >= 43(s+1)) at pos (s, b)
                ioS = ep.tile((1, 24 * 64), I32, name="ioS")
                nc.gpsimd.iota(ioS[:, :], [[43, 24], [0, 64]],
                               channel_multiplier=0)
                ioSf = ep.tile((1, 24 * 64), F32, name="ioSf")
                nc.vector.tensor_copy(ioSf[:, :], ioS[:, :])
                # ge1[g] = (s* >= g+1), ge2[g] = (s* >= g+2) with
                # s* = (len-1)//43; include G of seg s=g+1 iff ge2;
                # one-hot of s* group = ge1 - ge2
                lm44 = ep.tile((1, 24 * 64), F32, name="lm44")
                nc.vector.tensor_scalar(lm44[:, :], lenrep[:, :], 44.0,
                                        None, OP.subtract)
                ge1 = ep.tile((1, 24 * 64), F32, name="ge1")
                nc.vector.tensor_tensor(ge1[:, :], lm44[:, :], ioSf[:, :],
                                        OP.is_ge)
                lm87 = ep.tile((1, 24 * 64), F32, name="lm87")
                nc.vector.tensor_scalar(lm87[:, :], lenrep[:, :], 87.0,
                                        None, OP.subtract)
                ge2 = ep.tile((1, 24 * 64), F32, name="ge2")
                nc.vector.tensor_tensor(ge2[:, :], lm87[:, :], ioSf[:, :],
                                        OP.is_ge)
                ohsel = ep.tile((1, 24 * 64), F32, name="ohsel")
                nc.vector.tensor_tensor(ohsel[:, :], ge1[:, :], ge2[:, :],
                                        OP.subtract)
                tmp = ep.tile((1, 24 * 64), F32, name="tmp")
                nc.vector.tensor_tensor(tmp[:, :], ge2[:, :], Gall[:, :],
                                        OP.mult)
                nc.vector.tensor_tensor(comb[:, :], comb[:, :], tmp[:, :],
                                        OP.add)
                nc.vector.tensor_tensor(tmp[:, :], ohsel[:, :], s13[:, :],
                                        OP.mult)
                nc.vector.tensor_tensor(comb[:, :], comb[:, :], tmp[:, :],
                                        OP.subtract)
                # fold 24 groups -> 1
                for span in (768, 384, 192, 128, 64):
                    if span == 128:
                        nc.vector.tensor_tensor(comb[:, 0:64], comb[:, 0:64],
                                                comb[:, 128:192], OP.add)
                    else:
                        nc.vector.tensor_tensor(comb[:, 0:span],
                                                comb[:, 0:span],
                                                comb[:, span:2 * span],
                                                OP.add)
                # fwd = comb + lnsideS43 + len*(MU+ALPHA) - ALPHA
                fwd = ep.tile((1, Bc), F32, name="fwd")
                nc.vector.tensor_tensor(fwd[:, :], comb[:, 0:64],
                                        lnsideS43[:, :], OP.add)
                shifts = ep.tile((1, Bc), F32, name="shifts")
                nc.vector.tensor_scalar(shifts[:, :], lenrow[:, :],
                                        MU + ALPHA, -ALPHA, OP.mult, OP.add)
                nc.vector.tensor_tensor(fwd[:, :], fwd[:, :], shifts[:, :],
                                        OP.add)
                fwd_tot = ep.tile((1, 1), F32, name="fwd_tot")
                nc.vector.tensor_reduce(fwd_tot[:, :], fwd[:, :],
                                        mybir.AxisListType.X, OP.add)
                loss = ep.tile((1, 1), F32, name="loss")
                nc.vector.tensor_tensor(loss[:, :], fwd_tot[:, :],
                                        gold_ps[:, :], OP.subtract)
                nc.sync.dma_start(out_d.ap(), loss[:, :])
                if dbg:
                    gsb = ep.tile((1, 1), F32, name="gsb")
                    nc.scalar.copy(gsb[:, :], gold_ps[:, :])
                    nc.sync.dma_start(dbg_d.ap()[0:1, :], fwd[:, :])
                    nc.sync.dma_start(dbg_d.ap()[1:2, :], lnsideS43[:, :])
                    nc.sync.dma_start(dbg_d.ap()[2:3, :], lenrow[:, :])
                    nc.sync.dma_start(dbg_d.ap()[3:4, :], comb[:, 0:64])
                    nc.sync.dma_start(dbg_d.ap()[4:5, 0:1], gsb[:, :])
                    nc.sync.dma_start(dbg_d.ap()[5:6, :],
                                      comb[:, 0:64])
                    s44f = ep.tile((1, Bc), F32, name="s44f")
                    nc.vector.tensor_copy(s44f[:, :], sums_side44[:, :])
                    nc.sync.dma_start(dbg_d.ap()[6:7, :], s44f[:, :])
                    nc.sync.dma_start(dbg_d.ap()[7:8, :], lenrep[:, 0:64])

    nc.compile()
    return nc


def shard_inputs(feats, transitions, start_transitions, end_transitions,
                 tags, mask, n_cores=N_CORES):
    feats = np.ascontiguousarray(np.asarray(feats, dtype=np.float32))
    transitions = np.ascontiguousarray(
        np.asarray(transitions, dtype=np.float32))
    start_transitions = np.ascontiguousarray(
        np.asarray(start_transitions, dtype=np.float32))
    end_transitions = np.ascontiguousarray(
        np.asarray(end_transitions, dtype=np.float32))
    tags = np.ascontiguousarray(np.asarray(tags).astype(np.int32))
    mask = np.ascontiguousarray(np.asarray(mask).astype(np.int32))
    Bc = feats.shape[0] // n_cores
    in_maps = []
    for c in range(n_cores):
        s = slice(c * Bc, (c + 1) * Bc)
        in_maps.append({
            "feats": feats[s],
            "trans": transitions,
            "start": start_transitions,
            "end": end_transitions,
            "tags": tags[s],
            "mask": mask[s],
        })
    return in_maps, feats.shape


def kernel(feats, transitions, start_transitions, end_transitions, tags,
           mask, **_ignored):
    in_maps, _ = shard_inputs(
        feats, transitions, start_transitions, end_transitions, tags, mask)
    nc = build_program()
    res = run_bass_kernel_spmd(nc, in_maps, core_ids=list(range(N_CORES)))
    total = sum(float(r["out"][0, 0]) for r in res.results)
    return np.float32(total)


# revision 41
# speedup vs baseline: 1.8907x; 1.0304x over previous
"""Trainium2 Bass kernel for CRF negative-log-likelihood loss.

nn_CRF (B=512, L=1024, T=48), data-parallel over 8 NeuronCores (Bc=64
rows per core); host sums the 8 scalar partials.

Design (v2, segmented scan):
  Forward (partition function): the linear-domain scan
  A_t = (E^T A_{t-1}) * F_t (E = exp(trans - log T), F = exp(feat - MU))
  is split into NSEG=24 time segments of SEGLEN=43 steps with W=12
  warmup steps each (Hilbert-metric contraction of E makes the
  direction forget its init in ~8 steps, and diagonal F scalings are
  Hilbert isometries, so a warm-started segment converges to the true
  forward direction up to a per-column scale).  No renormalisation is
  needed inside a segment: fp32/bf16 exponent range absorbs the drift,
  and per-segment log-gains telescope through boundary column sums.
  Segments are packed 2-per-partition-group x 6-per-column-group into
  C=2 independent chains of (112, 384) matmul+multiply steps, with an
  exact side chain (48, 64) covering t in [0, 56) to anchor the
  telescoped magnitude.  The stationary matrix carries extra columns
  that compute end-capture rows and column sums for free; those rows
  ride through the F-multiply (F rows 48-63 are 1.0 via natfb padding)
  and are staged to DRAM, reloaded in (slot, seg*batch) layout, and
  selected by per-row length indicators.
  F tiles are produced by XBAR DMA transposes (128-source-column tiles)
  of a pre-exponentiated, 64-element-padded bf16 copy of feats (natfb),
  so the PE does no transposes and the ACT does no PSUM evacuation.
  Gold (numerator): one-hot tiles (bf16 tensor_scalar is_equal) over
  127-step chunks; bigram counts via offset-partition matmuls
  C += ohu[0:127]^T @ ohu[1:128] (mask baked into tags, so the pair
  weight mask_t*mask_{t+1} = mask_{t+1} is automatic); feature gathers
  via fused scalar_tensor_tensor ops split across DVE and Pool.
"""

import math

import numpy as np

import concourse.bacc as bacc
import concourse.mybir as mybir
import concourse.tile as tile
from concourse.bass import AP
from concourse.bass_utils import run_bass_kernel_spmd

F32 = mybir.dt.float32
BF16 = mybir.dt.bfloat16
I32 = mybir.dt.int32
AF = mybir.ActivationFunctionType
OP = mybir.AluOpType

B_FULL = 512
N_CORES = 8
BC = B_FULL // N_CORES  # 64
L_FULL = 1024
T = 48

MU = 0.51
ALPHA = math.log(T)
SEGLEN = 43
NSEG = 24          # segments s = 1..24, seg s main range [43s, 43s+43)
W = 12             # warmup steps
SLOTS = 56         # tau = 0..55; mm steps tau = 1..56
NTB = 576          # natfb window width in t-slots
W0 = 0             # chain-0 window start (t)
W1 = 508           # chain-1 window start (t)
NCH = 6            # column groups (m) per chain
WCH = NCH * BC     # 384 = chain column width
GOLD_CK = 8        # 128-step gold chunks


def build_program(dbg=False):
    L = L_FULL
    Bc = BC
    nc = bacc.Bacc("TRN2", target_bir_lowering=False, debug=False)

    feats_d = nc.dram_tensor("feats", (Bc, L, T), F32, kind="ExternalInput")
    trans_d = nc.dram_tensor("trans", (T, T), F32, kind="ExternalInput")
    start_d = nc.dram_tensor("start", (T,), F32, kind="ExternalInput")
    end_d = nc.dram_tensor("end", (T,), F32, kind="ExternalInput")
    tags_d = nc.dram_tensor("tags", (Bc, L), I32, kind="ExternalInput")
    mask_d = nc.dram_tensor("mask", (Bc, L), I32, kind="ExternalInput")
    out_d = nc.dram_tensor("out", (1, 1), F32, kind="ExternalOutput")
    dbg_d = (nc.dram_tensor("dbg", (8, Bc), F32, kind="ExternalOutput")
             if dbg else None)

    feats_flat = feats_d.ap().rearrange("b l t -> b (l t)")

    with tile.TileContext(nc) as tc:
        with (
            tc.tile_pool(name="const", bufs=1) as cp,
            tc.tile_pool(name="cps", bufs=1, space="PSUM") as cpp,
            tc.tile_pool(name="dramp", bufs=1, space="DRAM") as dp,
        ):
            # ---------------- constants ----------------
            iota48i = cp.tile((128, T), I32)
            nc.gpsimd.iota(iota48i[:, :], [[1, T]], channel_multiplier=0)
            iota48f = cp.tile((128, T), F32)
            nc.vector.tensor_copy(iota48f[:, :], iota48i[:, :])
            iota48b = cp.tile((128, T), BF16)
            nc.vector.tensor_copy(iota48b[:, :], iota48i[:, :])

            iota64i = cp.tile((64, 64), I32)
            nc.gpsimd.iota(iota64i[:, :], [[1, 64]], channel_multiplier=0)
            iotaPi = cp.tile((64, 1), I32)
            nc.gpsimd.iota(iotaPi[:, :], [[1, 1]], channel_multiplier=1)
            iota64f = cp.tile((64, 64), F32)
            nc.vector.tensor_copy(iota64f[:, :], iota64i[:, :])
            iotaPf = cp.tile((64, 1), F32)
            nc.vector.tensor_copy(iotaPf[:, :], iotaPi[:, :])
            identMf = cp.tile((64, 64), F32)
            nc.vector.tensor_scalar(
                identMf[:, :], iota64f[:, :], iotaPf[:, :], None, OP.is_equal)
            identMb = cp.tile((64, 64), BF16)
            nc.vector.tensor_copy(identMb[:, :], identMf[:, :])

            iotaLf = cp.tile((Bc, L), F32)

            ones128f = cp.tile((128, 1), F32)
            nc.vector.memset(ones128f[:, :], 1.0)
            ones128b = cp.tile((128, 1), BF16)
            nc.vector.memset(ones128b[:, :], 1.0)

            bias_mu = cp.tile((128, 1), F32)
            nc.vector.memset(bias_mu[:, :], -MU)
            bias_a = cp.tile((T, 1), F32)
            nc.vector.memset(bias_a[:, :], -ALPHA)

            # ---------------- params ----------------
            trans_sb = cp.tile((T, T), F32)
            nc.sync.dma_start(trans_sb[:, :], trans_d.ap())
            e_f32 = cp.tile((T, T), F32)
            nc.scalar.activation(e_f32[:, :], trans_sb[:, :], AF.Exp,
                                 bias=bias_a[:, :])
            e_b = cp.tile((T, T), BF16)
            nc.vector.tensor_copy(e_b[:, :], e_f32[:, :])

            end_sb = cp.tile((T, 1), F32)
            nc.sync.dma_start(end_sb[:, :], end_d.ap().unsqueeze(1))
            expend_f = cp.tile((T, 1), F32)
            nc.scalar.activation(expend_f[:, :], end_sb[:, :], AF.Exp)
            expend_b = cp.tile((T, 1), BF16)
            nc.vector.tensor_copy(expend_b[:, :], expend_f[:, :])

            start_sb = cp.tile((T, 1), F32)
            nc.sync.dma_start(start_sb[:, :], start_d.ap().unsqueeze(1))
            expstart = cp.tile((T, 1), F32)
            nc.scalar.activation(expstart[:, :], start_sb[:, :], AF.Exp)

            startbc = cp.tile((Bc, T), F32)
            nc.sync.dma_start(
                startbc[:, :], start_d.ap().unsqueeze(0).partition_broadcast(Bc))
            endbc = cp.tile((Bc, T), F32)
            nc.sync.dma_start(
                endbc[:, :], end_d.ap().unsqueeze(0).partition_broadcast(Bc))

            # main stationary (112, 112):
            #  rows 0-47 (block A), rows 64-111 (block B), rows 48-63 zero
            #  cols 0-47 = E(A), 64-111 = E(B), 48 = capA, 49 = capB,
            #  50 = sumA, 51 = sumB, 52-63 zero
            s_main = cp.tile((112, 112), BF16)
            nc.vector.memset(s_main[:, :], 0.0)
            nc.vector.tensor_copy(s_main[0:T, 0:T], e_b[:, :])
            nc.sync.dma_start(s_main[64:112, 64:112], e_b[:, :])
            nc.vector.tensor_copy(s_main[0:T, 48:49], expend_b[:, :])
            nc.sync.dma_start(s_main[64:112, 49:50], expend_b[:, :])
            nc.vector.memset(s_main[0:T, 50:51], 1.0)
            nc.vector.memset(s_main[64:112, 51:52], 1.0)

            # side stationary (48, 50): cols 0-47 E, 48 = cap, 49 = sum
            s_side = cp.tile((T, 50), BF16)
            nc.vector.tensor_copy(s_side[:, 0:T], e_b[:, :])
            nc.vector.tensor_copy(s_side[:, 48:49], expend_b[:, :])
            nc.vector.memset(s_side[:, 49:50], 1.0)

            # ---------------- tags / mask prep ----------------
            prep_scope = tc.tile_pool(name="prepsb", bufs=1)
            prp = prep_scope.__enter__()
            iotaLi = prp.tile((Bc, L), I32)
            nc.gpsimd.iota(iotaLi[:, :], [[1, L]], channel_multiplier=0)
            nc.vector.tensor_copy(iotaLf[:, :], iotaLi[:, :])
            tags_i = prp.tile((Bc, L), I32)
            nc.sync.dma_start(tags_i[:, :], tags_d.ap())
            tagsf = cp.tile((Bc, L), F32)
            nc.vector.tensor_copy(tagsf[:, :], tags_i[:, :])
            mask_i = prp.tile((Bc, L), I32)
            nc.sync.dma_start(mask_i[:, :], mask_d.ap())
            maskf = cp.tile((Bc, L), F32)
            nc.vector.tensor_copy(maskf[:, :], mask_i[:, :])
            tagsmb = prp.tile((Bc, L), BF16)
            moff = prp.tile((Bc, L), F32)
            nc.vector.tensor_scalar(moff[:, :], maskf[:, :], -100.0, 100.0,
                                    OP.mult, OP.add)
            tagsm_f = prp.tile((Bc, L), F32)
            nc.vector.tensor_tensor(tagsm_f[:, :], tagsf[:, :], moff[:, :],
                                    OP.add)
            nc.vector.tensor_copy(tagsmb[:, :], tagsm_f[:, :])

            tagsmSb = prp.tile((Bc, L), BF16)
            nc.vector.memset(tagsmSb[:, :], 100.0)
            nc.vector.tensor_copy(tagsmSb[:, 0:L - 1], tagsm_f[:, 1:L])

            lenb = cp.tile((Bc, 1), F32)
            nc.vector.tensor_reduce(lenb[:, :], maskf[:, :],
                                    mybir.AxisListType.X, OP.add)

            # transposed masked tags: 9 tiles (128, 64), 127-stride chunks
            tagsTm = []
            tagsTmS = []
            with tc.tile_pool(name="prepps", bufs=2, space="PSUM") as ppp:
                for k in range(GOLD_CK):
                    ps = ppp.tile((128, Bc), BF16, name=f"tps_{k}", tag="tps",
                                  bufs=2)
                    nc.tensor.transpose(ps[:, :],
                                        tagsmb[:, 128 * k:128 * (k + 1)],
                                        identMb[:, :])
                    tt = cp.tile((128, Bc), BF16, name=f"tagsTm_{k}")
                    nc.vector.tensor_copy(tt[:, :], ps[:, :])
                    tagsTm.append(tt)
                    ps2 = ppp.tile((128, Bc), BF16, name=f"tps2_{k}",
                                   tag="tps", bufs=2)
                    nc.tensor.transpose(ps2[:, :],
                                        tagsmSb[:, 128 * k:128 * (k + 1)],
                                        identMb[:, :])
                    tt2 = cp.tile((128, Bc), BF16, name=f"tagsTmS_{k}")
                    nc.vector.tensor_copy(tt2[:, :], ps2[:, :])
                    tagsTmS.append(tt2)
                # len row (1, 64) via transpose
                lps = ppp.tile((1, Bc), F32, name="lps", tag="lps", bufs=1)
                nc.tensor.transpose(lps[:, :], lenb[:, :], identMf[:, :])
                lenrow = cp.tile((1, Bc), F32)
                nc.vector.tensor_copy(lenrow[:, :], lps[:, :])
            prep_scope.__exit__(None, None, None)

            # (natfb/natfbS/A tiles are allocated inside the scan scope
            # below so their SBUF frees before the end phase)
            # ---------------- natfb: padded exp'd bf16 feats ----------------
            # (128, 56*12*64): row c*64+b holds chain c; column layout
            # (tau*12 + strip)*64 + jj with strip = m*2 + tp, so each
            # XBAR transpose slab input is CONTIGUOUS and 128-element
            # source groups give partitions tp*64 + jj.  jj 48-63 = 1.0
            # (become the F=1 ride-through rows after transpose).
            bigp_scope = tc.tile_pool(name="bigp", bufs=1)
            bigp = bigp_scope.__enter__()
            natfb = bigp.tile((128, SLOTS * 12 * 64), BF16)
            nc.vector.memset(
                natfb[:, :].rearrange("p (ts jj) -> p ts jj", ts=SLOTS * 12,
                                      jj=64)[:, :, T:64], 1.0)
            # invalid tails of the two clipped strips (chain 1, s=23, 24)
            # strip (tp=1, m=4) -> strip idx 9: slots tau >= 47 invalid
            # strip (tp=1, m=5) -> strip idx 11: slots tau >= 4 invalid
            nc.vector.memset(
                natfb[64:128, :].rearrange(
                    "p (t s jj) -> p t s jj", t=SLOTS, s=12,
                    jj=64)[:, 47:SLOTS, 9, 0:T], 1.0)
            nc.vector.memset(
                natfb[64:128, :].rearrange(
                    "p (t s jj) -> p t s jj", t=SLOTS, s=12,
                    jj=64)[:, 4:SLOTS, 11, 0:T], 1.0)
            # side-chain feats: plain t-slot layout, t in [0, 56)
            natfbS = bigp.tile((Bc, SLOTS * 64), BF16)
            nc.vector.memset(
                natfbS[:, :].rearrange("p (t jj) -> p t jj", t=SLOTS,
                                       jj=64)[:, :, T:64], 1.0)

            # stage DRAM: per chain (4, 57*384) bf16
            stage_dr = [dp.tile((4, 57 * WCH), BF16, name=f"stage_{c}")
                        for c in range(2)]

            with (
                tc.tile_pool(name="chkp", bufs=2) as chp,
                tc.tile_pool(name="fslab", bufs=2) as fsp,
                tc.tile_pool(name="fside", bufs=2) as fsdp,
                tc.tile_pool(name="scanps", bufs=3, space="PSUM") as sps,
                tc.tile_pool(name="scanps2", bufs=3, space="PSUM") as sps2,
                tc.tile_pool(name="sideps", bufs=1, space="PSUM") as sdps,
                tc.tile_pool(name="ohp", bufs=4) as ohp,
                tc.tile_pool(name="bounce", bufs=1) as bpp,
                tc.tile_pool(name="fgp", bufs=2) as fgp,
                tc.tile_pool(name="scrp", bufs=2) as scrp,
            ):
                # ---- feats strip DMAs + exp into natfb ----
                natfb4 = natfb[:, :].rearrange(
                    "p (t s jj) -> p t s jj", t=SLOTS, s=12, jj=64)
                natfb4c1 = natfb[64:128, :].rearrange(
                    "p (t s jj) -> p t s jj", t=SLOTS, s=12, jj=64)

                HS = SLOTS // 2  # 28-slot half strips

                def emit_strip(tp, m):
                    # strip = m*2 + tp; seg s_c = 12c + 6tp + m + 1;
                    # t range [43s - 12, 43s + 44)
                    strip = m * 2 + tp
                    s0 = 6 * tp + m + 1
                    s1 = s0 + 12
                    nvalid1 = min(SLOTS, max(0, L - (SEGLEN * s1 - 12)))
                    for h in range(2):
                        t0 = SEGLEN * s0 - 12 + h * HS
                        tsl = slice(h * HS, (h + 1) * HS)
                        if nvalid1 == SLOTS:
                            ch = chp.tile((128, HS * T), F32, name="natf")
                            in_ap = AP(feats_flat.tensor, t0 * T,
                                       [[516 * T, 2], [L * T, Bc],
                                        [1, HS * T]])
                            nc.sync.dma_start(
                                ch[:, :].rearrange("p (a b) -> p a b",
                                                   a=1, b=HS * T), in_ap)
                            nc.scalar.activation(
                                natfb4[:, tsl, strip, 0:T],
                                ch[:, :], AF.Exp, bias=bias_mu[:, :])
                        else:
                            ch = chp.tile((128, HS * T), F32, name="natf")
                            in_ap = AP(feats_flat.tensor, t0 * T,
                                       [[L * T, Bc], [1, HS * T]])
                            nc.sync.dma_start(ch[0:Bc, :], in_ap)
                            nc.scalar.activation(
                                natfb4[0:Bc, tsl, strip, 0:T],
                                ch[0:Bc, :], AF.Exp, bias=bias_mu[0:Bc, :])
                            nv = min(max(nvalid1 - h * HS, 0), HS)
                            if nv > 0:
                                ch2 = chp.tile((128, HS * T), F32,
                                               name="natf")
                                in2 = AP(feats_flat.tensor,
                                         (SEGLEN * s1 - 12 + h * HS) * T,
                                         [[L * T, Bc], [1, nv * T]])
                                nc.sync.dma_start(ch2[0:Bc, 0:nv * T], in2)
                                nc.scalar.activation(
                                    natfb4c1[:, h * HS:h * HS + nv,
                                             strip, 0:T],
                                    ch2[0:Bc, 0:nv * T], AF.Exp,
                                    bias=bias_mu[0:Bc, :])

                # side strip first (unblocks the side chain)
                for h in range(2):
                    HSs = SLOTS // 2
                    chS = chp.tile((128, HSs * T), F32, name="natf")
                    nc.sync.dma_start(
                        chS[0:Bc, :],
                        AP(feats_flat.tensor, h * HSs * T,
                           [[L * T, Bc], [1, HSs * T]]))
                    nc.scalar.activation(
                        natfbS[:, :].rearrange(
                            "p (t jj) -> p t jj", t=SLOTS,
                            jj=64)[:, h * HSs:(h + 1) * HSs, 0:T],
                        chS[0:Bc, :], AF.Exp, bias=bias_mu[0:Bc, :])
                for m in range(NCH):
                    for tp in range(2):
                        emit_strip(tp, m)

                # ---- side chain (exact, t in [0, 56]) ----
                # F side slabs: q covers tau in [8q, 8q+8)
                side_slabs = {}

                natfbS_t = natfbS[:, :].tensor

                def emit_side_slab(q):
                    sl = fsdp.tile((128, 4 * 64), BF16, name="fside")
                    in_ap = AP(natfbS_t, 8 * q * 64,
                               [[SLOTS * 64, Bc], [1, 512]])
                    nc.scalar.dma_start_transpose(
                        sl[:, :].rearrange("p (e b) -> p e b", e=4, b=64),
                        in_ap)
                    side_slabs[q] = sl

                def side_f(tau):
                    sl = side_slabs[tau // 8]
                    p0 = (tau % 2) * 64
                    c0 = ((tau // 2) % 4) * 64
                    return sl[p0:p0 + 50, c0:c0 + 64]

                emit_side_slab(0)
                emit_side_slab(1)

                side_pool = tc.tile_pool(name="sidea", bufs=3)
                sap = side_pool.__enter__()
                a_side = sap.tile((50, Bc), BF16, name="a_side")
                # A_side(0) = exp(start) * F_0  (rows 48-49 will be junk)
                nc.vector.memset(a_side[32:50, :], 1.0)
                nc.vector.tensor_scalar(a_side[0:T, :],
                                        side_slabs[0][0:T, 0:64],
                                        expstart[:, :], None, OP.mult)
                lnsideS43 = cp.tile((1, Bc), F32)
                sums_side44 = cp.tile((1, Bc), BF16)

                for tau in range(1, 45):
                    if tau % 8 == 1 and tau // 8 + 2 <= 5:
                        emit_side_slab(tau // 8 + 2)
                    ps = sdps.tile((50, Bc), F32, name="side_ps")
                    nc.tensor.matmul(ps[:, :], s_side[:, :], a_side[0:T, :],
                                     start=True, stop=True,
                                     skip_group_check=True)
                    a_new = sap.tile((50, Bc), BF16, name="a_side")
                    nc.vector.tensor_tensor(a_new[:, :], ps[:, :],
                                            side_f(tau), OP.mult)
                    if tau == 44:
                        nc.sync.dma_start(sums_side44[:, :],
                                          a_new[49:50, :])
                    a_side = a_new
                nc.scalar.activation(lnsideS43[:, :], sums_side44[:, :],
                                     AF.Ln)
                side_pool.__exit__(None, None, None)

                # ---- main F slabs ----
                main_slabs = {}
                natfb_t = natfb[:, :].tensor

                def emit_main_slab(c, q):
                    sl = fsp.tile((128, 8 * WCH), BF16, name="fslab")
                    if c == 0:
                        in_ap = AP(natfb_t, q * 8 * 768,
                                   [[SLOTS * 12 * 64, Bc], [1, 8 * 768]])
                    else:
                        # XBAR input must start at partition 0: bounce
                        # the chain-1 span down via an SBUF DMA first
                        bt = bpp.tile((Bc, 8 * 768), BF16, name="bounce")
                        nc.sync.dma_start(
                            bt[:, :],
                            AP(natfb_t,
                               64 * (SLOTS * 12 * 64) + q * 8 * 768,
                               [[SLOTS * 12 * 64, Bc], [1, 8 * 768]]))
                        in_ap = bt[:, :]
                    teng = nc.scalar if (c + q) % 2 == 0 else nc.sync
                    teng.dma_start_transpose(
                        sl[:, :].rearrange("p (e b) -> p e b", e=8 * NCH,
                                           b=64),
                        in_ap)
                    main_slabs[(c, q)] = sl

                for c in range(2):
                    emit_main_slab(c, 0)
                    emit_main_slab(c, 1)

                # ---- A ping-pong tiles ----
                app = [[bigp.tile((112, 8 * WCH), BF16, name=f"A_{c}_{i}")
                        for i in range(2)] for c in range(2)]
                for c in range(2):
                    t0 = app[c][0]
                    nc.vector.memset(t0[0:64, 0:WCH], 0.0)
                    nc.vector.memset(t0[0:52, 0:WCH], 1.0)
                    nc.vector.memset(t0[64:112, 0:WCH], 1.0)

                # ---- gold work generator (interleaved) ----
                c_ps = cpp.tile((T, T), F32, name="c_ps")
                feat_acc = cp.tile((128, 64), F32)
                nc.vector.memset(feat_acc[:, :], 0.0)
                # b-major iota: val[p, b*48+j] = j
                iota384i = cp.tile((128, 384), I32)
                nc.gpsimd.iota(iota384i[:, :], [[0, 8], [1, T]],
                               channel_multiplier=0)
                iota384b = cp.tile((128, 384), BF16)
                nc.vector.tensor_copy(iota384b[:, :], iota384i[:, :])

                gold_units = [(o, k) for o in range(8)
                              for k in range(GOLD_CK)]
                n_units = len(gold_units)
                gold_pos = [0]
                first_c = [True]

                def emit_gold(n):
                    for _ in range(n):
                        u = gold_pos[0]
                        if u >= n_units:
                            return
                        o, k = gold_units[u]
                        fg8 = fgp.tile((128, 384), F32, name="fg8")
                        in_ap = AP(feats_flat.tensor,
                                   8 * o * L * T + 128 * k * T,
                                   [[T, 128], [L * T, 8], [1, T]])
                        nc.gpsimd.dma_start(fg8[:, :], in_ap)
                        ohu8 = ohp.tile((128, 384), BF16, name="ohu8")
                        tu = tagsTm[k][:, :].tensor
                        nc.vector.tensor_tensor(
                            ohu8[:, :], iota384b[:, :],
                            AP(tu, 8 * o, [[Bc, 128], [1, 8], [0, T]]),
                            OP.is_equal)
                        ohs8 = ohp.tile((128, 384), BF16, name="ohs8")
                        ts_ = tagsTmS[k][:, :].tensor
                        nc.vector.tensor_tensor(
                            ohs8[:, :], iota384b[:, :],
                            AP(ts_, 8 * o, [[Bc, 128], [1, 8], [0, T]]),
                            OP.is_equal)
                        for b in range(8):
                            nc.tensor.matmul(
                                c_ps[:, :],
                                ohu8[:, b * T:(b + 1) * T],
                                ohs8[:, b * T:(b + 1) * T],
                                start=first_c[0], stop=False,
                                skip_group_check=True)
                            first_c[0] = False
                        scr = scrp.tile((128, 384), F32, name="scr",
                                        tag="scr")
                        nc.vector.scalar_tensor_tensor(
                            scr[:, :], ohu8[:, :], 1.0, fg8[:, :],
                            OP.mult, OP.mult,
                            accum_out=feat_acc[:, u:u + 1])
                        gold_pos[0] += 1

                # ---- main scan ----
                def a_slice(c, tau):
                    return app[c][(tau // 8) % 2][:, (tau % 8) * WCH:
                                                  (tau % 8 + 1) * WCH]

                def f_slice(c, tau):
                    # step 56 only needs the F=1 ride-through rows; reuse
                    # slot 55 (A rows 0-111 of state 56 are never used)
                    tau = min(tau, SLOTS - 1)
                    sl = main_slabs[(c, tau // 8)]
                    return sl[0:112, (tau % 8) * WCH:(tau % 8 + 1) * WCH]

                emit_gold(6)
                for tau in range(1, SLOTS + 1):
                    if tau % 8 == 1:
                        q = tau // 8 + 2
                        if q <= 6:
                            for c in range(2):
                                emit_main_slab(c, q)
                    for c in range(2):
                        pool = sps if c == 0 else sps2
                        ps = pool.tile((112, WCH), F32, name=f"mm_{c}")
                        nc.tensor.matmul(ps[:, :], s_main[:, :],
                                         a_slice(c, tau - 1)[0:112, :],
                                         start=True, stop=True,
                                         skip_group_check=True)
                        nc.vector.tensor_tensor(a_slice(c, tau), ps[:, :],
                                                f_slice(c, tau), OP.mult)
                    if tau % 8 == 7 or tau == SLOTS:
                        pass
                    if tau % 8 == 0:
                        q = tau // 8 - 1
                        for c in range(2):
                            nc.sync.dma_start(
                                stage_dr[c][0:4,
                                            q * 8 * WCH:(q + 1) * 8 * WCH],
                                app[c][q % 2][48:52, :])
                    emit_gold(2)
                # final partial block: slot 56 = A(56)
                for c in range(2):
                    nc.sync.dma_start(
                        stage_dr[c][0:4, 56 * WCH:57 * WCH],
                        app[c][1][48:52, 0:WCH])
                emit_gold(n_units)
                zrow = cp.tile((1, T), BF16)
                nc.vector.memset(zrow[:, :], 0.0)
                nc.tensor.matmul(c_ps[:, :], zrow[:, :], zrow[:, :],
                                 start=False, stop=True,
                                 skip_group_check=True)

                # ---- gold misc terms (start, end-transitions) ----
                misc_acc = cp.tile((Bc, 2), F32)
                scrb = scrp.tile((Bc, T), F32, name="scrb", tag="scrb")
                nc.vector.scalar_tensor_tensor(
                    scrb[:, :], iota48f[0:Bc, :], tagsf[:, 0:1],
                    startbc[:, :], OP.is_equal, OP.mult,
                    accum_out=misc_acc[:, 0:1])
                lm1 = cp.tile((Bc, 1), F32)
                nc.vector.tensor_scalar(lm1[:, :], lenb[:, :], 1.0, None,
                                        OP.subtract)
                scrL = cp.tile((Bc, L), F32)
                lt = cp.tile((Bc, 1), F32)
                nc.vector.scalar_tensor_tensor(
                    scrL[:, :], iotaLf[:, :], lm1[:, :], tagsf[:, :],
                    OP.is_equal, OP.mult, accum_out=lt[:, :])
                scrb3 = scrp.tile((Bc, T), F32, name="scrb3", tag="scrb")
                nc.vector.scalar_tensor_tensor(
                    scrb3[:, :], iota48f[0:Bc, :], lt[:, :], endbc[:, :],
                    OP.is_equal, OP.mult, accum_out=misc_acc[:, 1:2])

            bigp_scope.__exit__(None, None, None)

            # =============== end phase ===============
            with (
                tc.tile_pool(name="endp", bufs=1) as ep,
                tc.tile_pool(name="endps", bufs=1, space="PSUM") as epp,
                tc.tile_pool(name="endps2", bufs=2, space="PSUM") as epp2,
            ):
                # gold assembly
                gold_ps = epp.tile((1, 1), F32, name="gold_ps")
                scrT = ep.tile((T, T), F32, name="scrT")
                cacc = ep.tile((T, 1), F32, name="cacc")
                nc.vector.tensor_tensor(scrT[:, :], c_ps[:, :],
                                        trans_sb[:, :], OP.mult)
                nc.vector.tensor_reduce(cacc[:, :], scrT[:, :],
                                        mybir.AxisListType.X, OP.add)
                nc.tensor.matmul(gold_ps[:, :], ones128f[0:T, :], cacc[:, :],
                                 start=True, stop=False,
                                 skip_group_check=True)
                fred = ep.tile((128, 1), F32, name="fred")
                nc.vector.tensor_reduce(fred[:, :], feat_acc[:, :],
                                        mybir.AxisListType.X, OP.add)
                nc.tensor.matmul(gold_ps[:, :], ones128f[:, :],
                                 fred[:, :], start=False, stop=False,
                                 skip_group_check=True)
                mred = ep.tile((Bc, 1), F32, name="mred")
                nc.vector.tensor_reduce(mred[:, :], misc_acc[:, :],
                                        mybir.AxisListType.X, OP.add)
                nc.tensor.matmul(gold_ps[:, :], ones128f[0:Bc, :],
                                 mred[:, :], start=False, stop=True,
                                 skip_group_check=True)

                # reload caps/sums: (57, 768) per chain
                capsre, sumsre, capsLn, sumsLn = [], [], [], []
                for c in range(2):
                    cr = ep.tile((57, 2 * WCH), BF16, name=f"capsre_{c}")
                    sr = ep.tile((57, 2 * WCH), BF16, name=f"sumsre_{c}")
                    st_t = stage_dr[c][:, :].tensor
                    nc.sync.dma_start(
                        cr[:, :], AP(st_t, 0,
                                     [[WCH, 57], [57 * WCH, 2],
                                      [64, NCH], [1, 64]]))
                    nc.sync.dma_start(
                        sr[:, :], AP(st_t, 2 * 57 * WCH,
                                     [[WCH, 57], [57 * WCH, 2],
                                      [64, NCH], [1, 64]]))
                    cl = ep.tile((57, 2 * WCH), F32, name=f"capsLn_{c}")
                    sl_ = ep.tile((57, 2 * WCH), F32, name=f"sumsLn_{c}")
                    nc.scalar.activation(cl[:, :], cr[:, :], AF.Ln)
                    nc.scalar.activation(sl_[:, :], sr[:, :], AF.Ln)
                    capsre.append(cr)
                    sumsre.append(sr)
                    capsLn.append(cl)
                    sumsLn.append(sl_)

                # lenrep (1, 1536) f32
                lenrep = ep.tile((1, 24 * 64), F32, name="lenrep")
                nc.vector.tensor_copy(lenrep[:, 0:64], lenrow[:, :])
                for w_ in (64, 128, 256, 512):
                    nc.vector.tensor_copy(lenrep[:, w_:2 * w_],
                                          lenrep[:, 0:w_])
                nc.vector.tensor_copy(lenrep[:, 1024:1536],
                                      lenrep[:, 0:512])

                # per-chain sigma indicator + capture select
                ones_row = ep.tile((1, 64), F32, name="ones_row")
                nc.vector.memset(ones_row[:, :], 1.0)
                comb = ep.tile((1, 24 * 64), F32, name="comb")
                for c in range(2):
                    io = ep.tile((57, 2 * WCH), I32, name=f"indio_{c}")
                    nc.gpsimd.iota(io[:, :], [[258, 2], [43, NCH], [0, 64]],
                                   channel_multiplier=1)
                    iof = ep.tile((57, 2 * WCH), F32,
                                  name=f"indiof_{c}")
                    nc.vector.tensor_copy(iof[:, :], io[:, :])
                    nc.vector.memset(iof[0:13, :], -9999.0)
                    if c == 0:
                        negrow = ep.tile((1, 2 * WCH), F32, name="negrow")
                        nc.vector.memset(negrow[:, :], -9999.0)
                    nc.sync.dma_start(iof[56:57, :], negrow[:, :])
                    lr_c = ep.tile((1, 2 * WCH), F32, name=f"lrc_{c}")
                    nc.vector.tensor_scalar(
                        lr_c[:, :], lenrep[:, 0:2 * WCH],
                        float(31 + 516 * c), None, OP.subtract)
                    for h in range(2):
                        hs = slice(h * WCH, (h + 1) * WCH)
                        lps_c = epp2.tile((57, WCH), F32,
                                          name=f"lps_{c}_{h}", tag="lps",
                                          bufs=2)
                        nc.tensor.matmul(lps_c[:, :], ones_row[0:1, 0:57],
                                         lr_c[:, hs], start=True, stop=True,
                                         skip_group_check=True)
                        ind = ep.tile((57, WCH), F32,
                                      name=f"ind_{c}_{h}")
                        nc.vector.tensor_tensor(ind[:, :], iof[:, hs],
                                                lps_c[:, :], OP.is_equal)
                        pr = ep.tile((57, WCH), F32,
                                      name=f"pr_{c}_{h}")
                        nc.vector.tensor_tensor(pr[:, :], capsLn[c][:, hs],
                                                ind[:, :], OP.mult)
                        fsel = epp2.tile((1, WCH), F32,
                                         name=f"fsel_{c}_{h}", tag="fsel",
                                         bufs=2)
                        nc.tensor.matmul(fsel[:, :], ones128f[0:57, :],
                                         pr[:, :], start=True, stop=True,
                                         skip_group_check=True)
                        nc.vector.tensor_copy(
                            comb[:, c * 768 + h * WCH:
                                 c * 768 + (h + 1) * WCH], fsel[:, :])
                Gall = ep.tile((1, 24 * 64), F32, name="Gall")
                s13 = ep.tile((1, 24 * 64), F32, name="s13")
                s56 = ep.tile((1, 24 * 64), F32, name="s56")
                for c in range(2):
                    nc.sync.dma_start(s13[:, c * 768:(c + 1) * 768],
                                      sumsLn[c][13:14, :])
                    nc.sync.dma_start(s56[:, c * 768:(c + 1) * 768],
                                      sumsLn[c][56:57, :])
                nc.vector.tensor_tensor(Gall[:, :], s56[:, :], s13[:, :],
                                        OP.subtract)
                # ge[s] = (len-1 >= 43(s+1)) at pos (s, b)
                ioS = ep.tile((1, 24 * 64), I32, name="ioS")
                nc.gpsimd.iota(ioS[:, :], [[43, 24], [0, 64]],
                               channel_multiplier=0)
                ioSf = ep.tile((1, 24 * 64), F32, name="ioSf")
                nc.vector.tensor_copy(ioSf[:, :], ioS[:, :])
                # ge1[g] = (s* >= g+1), ge2[g] = (s* >= g+2) with
                # s* = (len-1)//43; include G of seg s=g+1 iff ge2;
                # one-hot of s* group = ge1 - ge2
                lm44 = ep.tile((1, 24 * 64), F32, name="lm44")
                nc.vector.tensor_scalar(lm44[:, :], lenrep[:, :], 44.0,
                                        None, OP.subtract)
                ge1 = ep.tile((1, 24 * 64), F32, name="ge1")
                nc.vector.tensor_tensor(ge1[:, :], lm44[:, :], ioSf[:, :],
                                        OP.is_ge)
                lm87 = ep.tile((1, 24 * 64), F32, name="lm87")
                nc.vector.tensor_scalar(lm87[:, :], lenrep[:, :], 87.0,
                                        None, OP.subtract)
                ge2 = ep.tile((1, 24 * 64), F32, name="ge2")
                nc.vector.tensor_tensor(ge2[:, :], lm87[:, :], ioSf[:, :],
                                        OP.is_ge)
                ohsel = ep.tile((1, 24 * 64), F32, name="ohsel")
                nc.vector.tensor_tensor(ohsel[:, :], ge1[:, :], ge2[:, :],
                                        OP.subtract)
                tmp = ep.tile((1, 24 * 64), F32, name="tmp")
                nc.vector.tensor_tensor(tmp[:, :], ge2[:, :], Gall[:, :],
                                        OP.mult)
                nc.vector.tensor_tensor(comb[:, :], comb[:, :], tmp[:, :],
                                        OP.add)
                nc.vector.tensor_tensor(tmp[:, :], ohsel[:, :], s13[:, :],
                                        OP.mult)
                nc.vector.tensor_tensor(comb[:, :], comb[:, :], tmp[:, :],
                                        OP.subtract)
                # fold 24 groups -> 1
                for span in (768, 384, 192, 128, 64):
                    if span == 128:
                        nc.vector.tensor_tensor(comb[:, 0:64], comb[:, 0:64],
                                                comb[:, 128:192], OP.add)
                    else:
                        nc.vector.tensor_tensor(comb[:, 0:span],
                                                comb[:, 0:span],
                                                comb[:, span:2 * span],
                                                OP.add)
                # fwd = comb + lnsideS43 + len*(MU+ALPHA) - ALPHA
                fwd = ep.tile((1, Bc), F32, name="fwd")
                nc.vector.tensor_tensor(fwd[:, :], comb[:, 0:64],
                                        lnsideS43[:, :], OP.add)
                shifts = ep.tile((1, Bc), F32, name="shifts")
                nc.vector.tensor_scalar(shifts[:, :], lenrow[:, :],
                                        MU + ALPHA, -ALPHA, OP.mult, OP.add)
                nc.vector.tensor_tensor(fwd[:, :], fwd[:, :], shifts[:, :],
                                        OP.add)
                fwd_tot = ep.tile((1, 1), F32, name="fwd_tot")
                nc.vector.tensor_reduce(fwd_tot[:, :], fwd[:, :],
                                        mybir.AxisListType.X, OP.add)
                loss = ep.tile((1, 1), F32, name="loss")
                nc.vector.tensor_tensor(loss[:, :], fwd_tot[:, :],
                                        gold_ps[:, :], OP.subtract)
                nc.sync.dma_start(out_d.ap(), loss[:, :])
                if dbg:
                    gsb = ep.tile((1, 1), F32, name="gsb")
                    nc.scalar.copy(gsb[:, :], gold_ps[:, :])
                    nc.sync.dma_start(dbg_d.ap()[0:1, :], fwd[:, :])
                    nc.sync.dma_start(dbg_d.ap()[1:2, :], lnsideS43[:, :])
                    nc.sync.dma_start(dbg_d.ap()[2:3, :], lenrow[:, :])
                    nc.sync.dma_start(dbg_d.ap()[3:4, :], comb[:, 0:64])
                    nc.sync.dma_start(dbg_d.ap()[4:5, 0:1], gsb[:, :])
                    nc.sync.dma_start(dbg_d.ap()[5:6, :],
                                      comb[:, 0:64])
                    s44f = ep.tile((1, Bc), F32, name="s44f")
                    nc.vector.tensor_copy(s44f[:, :], sums_side44[:, :])
                    nc.sync.dma_start(dbg_d.ap()[6:7, :], s44f[:, :])
                    nc.sync.dma_start(dbg_d.ap()[7:8, :], lenrep[:, 0:64])

    nc.compile()
    return nc


def shard_inputs(feats, transitions, start_transitions, end_transitions,
                 tags, mask, n_cores=N_CORES):
    feats = np.ascontiguousarray(np.asarray(feats, dtype=np.float32))
    transitions = np.ascontiguousarray(
        np.asarray(transitions, dtype=np.float32))
    start_transitions = np.ascontiguousarray(
        np.asarray(start_transitions, dtype=np.float32))
    end_transitions = np.ascontiguousarray(
        np.asarray(end_transitions, dtype=np.float32))
    tags = np.ascontiguousarray(np.asarray(tags).astype(np.int32))
    mask = np.ascontiguousarray(np.asarray(mask).astype(np.int32))
    Bc = feats.shape[0] // n_cores
    in_maps = []
    for c in range(n_cores):
        s = slice(c * Bc, (c + 1) * Bc)
        in_maps.append({
            "feats": feats[s],
            "trans": transitions,
            "start": start_transitions,
            "end": end_transitions,
            "tags": tags[s],
            "mask": mask[s],
        })
    return in_maps, feats.shape


def kernel(feats, transitions, start_transitions, end_transitions, tags,
           mask, **_ignored):
    in_maps, _ = shard_inputs(
        feats, transitions, start_transitions, end_transitions, tags, mask)
    nc = build_program()
    res = run_bass_kernel_spmd(nc, in_maps, core_ids=list(range(N_CORES)))
    total = sum(float(r["out"][0, 0]) for r in res.results)
    return np.float32(total)
